# revision 1
# baseline (speedup 1.0000x reference)
"""Trainium2 Bass kernel for nn_NUFFTLayerMultiChannelInitMixed.

Math: the reference's spread->FFT->filter->IFFT->energy pipeline is an exact
bilinear form in the (analytic) spectrum of the periodized Gaussians:

  ghat_n(k) = Cc * sum_{r in -1,0,1} exp(-tau (k+rM)^2) exp(-i (k+rM) x_n)

With alpha_n = cos(M x_n), beta_n = sin(M x_n), A = p_k + q_k alpha_n,
B = d_k beta_n (p/q/d from the alias weights), c = cos(k x_n), s = sin(k x_n):

  Re ghat = A c - B s,   -Im ghat = A s + B c,   |ghat|^2 = A^2 + B^2  (!)

Energy per channel i (filter F_i(k) = deconv^2 * mult_i, even in k):

  e_i[n] = pref * ( T_i[n] - self_i[n] )
  T_i[n] = sum_k w_k F_i (Re_n ReS + Im_n ImS)   (S = sum over points)
  self_i[n] = quadratic polynomial in alpha_n, beta_n (no trig!)

T_i reduces to two small matmul families against the [K, N] cos/sin
matrices (K = 128 truncation keeps rel err ~2e-5; the filter decays ~1/k^2).
Sharding: batch-parallel, 2 of 16 batches per core, no collectives.
"""

import numpy as np

try:
    import concourse.bass as bass
except ImportError:
    import sys
    sys.path.insert(0, "/opt/trn_rl_repo")
    import concourse.bass as bass

import concourse.bacc as bacc
import concourse.mybir as mybir
from concourse import tile
from concourse.bass_utils import run_bass_kernel_spmd

F32 = mybir.dt.float32
AF = mybir.ActivationFunctionType
ALU = mybir.AluOpType

M = 2001
L = 2.0 * np.pi
TAU = 12.0 * (L / (2.0 * np.pi * M)) ** 2
KTRUNC = 128
B_FULL, N = 16, 1024
NCORES = 8
BPC = B_FULL // NCORES  # batches per core
MAGIC = 12582912.0      # 1.5 * 2^23: (u + MAGIC) - MAGIC = round-to-nearest(u)
PI = float(np.pi)


def _host_constants(shift0, shift1, amp0, amp1):
    """fp64 host-side k-space constants -> packed [128, 12] table + scalars."""
    k = np.arange(KTRUNC, dtype=np.float64)
    tau = float(TAU)
    p = np.exp(-tau * k * k)
    apl = np.exp(-tau * (k + M) ** 2)
    amn = np.exp(-tau * (k - M) ** 2)
    q = apl + amn
    d = apl - amn
    Cc = (M / L) * np.sqrt(4.0 * np.pi * tau)
    deconv2 = (np.pi / tau) * np.exp(2.0 * tau * k * k)
    mult1 = float(amp0) * (4.0 * np.pi) / (k * k + (1.0 * float(shift0)) ** 2)
    mult2 = float(amp1) * (4.0 * np.pi) / (k * k + (0.5 * float(shift1)) ** 2)
    w = np.full(KTRUNC, 2.0)
    w[0] = 1.0
    scale = 1.0 / ((2.0 * np.pi * M / L) * (2.0 * np.pi))
    pref = scale * Cc * Cc / M
    wF1 = w * deconv2 * mult1
    wF2 = w * deconv2 * mult2

    cst = np.zeros((128, 12), dtype=np.float64)
    cst[:, 0] = p
    cst[:, 1] = q
    cst[:, 2] = -d          # dneg (for S_R)
    cst[:, 3] = d           # (for S_I)
    cst[:, 4] = pref * p * wF1
    cst[:, 5] = pref * q * wF1
    cst[:, 6] = pref * p * wF2
    cst[:, 7] = pref * q * wF2
    cst[:, 8] = pref * d * wF1
    cst[:, 9] = pref * d * wF2
    cst[:, 10] = -pref * d * wF1
    cst[:, 11] = -pref * d * wF2

    def selfsc(wF):
        return [float(pref * np.sum(wF * p * p)),
                float(pref * 2.0 * np.sum(wF * p * q)),
                float(pref * np.sum(wF * q * q)),
                float(pref * np.sum(wF * d * d))]

    return cst.astype(np.float32), selfsc(wF1), selfsc(wF2)


def _emit_batch(nc, tc, pools, b, t_in, out_t, kv, cst, ident, sc1, sc2):
    pc, wp, sp, ps_u, ps_t, ps_cs, ps_T = pools
    KT = KTRUNC

    # --- phase matrix u = k (x) t  (outer product on PE), then r = u - rni(u)
    trow = sp.tile([1, N], F32, tag="trow")
    nc.sync.dma_start(trow[:], t_in[b])
    t88 = sp.tile([128, 8], F32, tag="t88")
    nc.sync.dma_start(t88[:], t_in[b].rearrange("(j p) -> p j", p=128))

    u_ps = ps_u.tile([128, N], F32, tag="u")
    nc.tensor.matmul(u_ps[:, 0:512], kv[:], trow[:, 0:512], start=True, stop=True)
    nc.tensor.matmul(u_ps[:, 512:1024], kv[:], trow[:, 512:1024], start=True, stop=True)

    rni = wp.tile([128, N], F32, tag="rni")
    nc.vector.tensor_scalar(rni[:], u_ps[:], MAGIC, MAGIC, ALU.add, ALU.subtract)
    r = wp.tile([128, N], F32, tag="r")
    nc.vector.tensor_sub(r[:], u_ps[:], rni[:])

    # --- big trig matrices (layout [k-part, n-free]); true cos/sin values
    smat = wp.tile([128, N], F32, tag="smat")
    nc.scalar.activation(smat[:], r[:], AF.Sin, scale=2.0 * PI)
    h = wp.tile([128, N], F32, tag="h")
    nc.scalar.activation(h[:], r[:], AF.Sin, scale=PI)
    hh = wp.tile([128, N], F32, tag="hh")
    nc.vector.tensor_mul(hh[:], h[:], h[:])
    cmat = wp.tile([128, N], F32, tag="cmat")
    nc.vector.tensor_scalar(cmat[:], hh[:], 2.0, 1.0, ALU.mult, ALU.subtract)

    # --- alpha/beta = cos/sin(M x) in [128, 8] (n = 128*j + p)
    u8 = sp.tile([128, 8], F32, tag="u8")
    nc.vector.tensor_scalar_mul(u8[:], t88[:], float(M))
    rni8 = sp.tile([128, 8], F32, tag="rni8")
    nc.vector.tensor_scalar(rni8[:], u8[:], MAGIC, MAGIC, ALU.add, ALU.subtract)
    r8 = sp.tile([128, 8], F32, tag="r8")
    nc.vector.tensor_sub(r8[:], u8[:], rni8[:])
    beta = sp.tile([128, 8], F32, tag="beta")
    nc.scalar.activation(beta[:], r8[:], AF.Sin, scale=2.0 * PI)
    h8 = sp.tile([128, 8], F32, tag="h8")
    nc.scalar.activation(h8[:], r8[:], AF.Sin, scale=PI)
    hh8 = sp.tile([128, 8], F32, tag="hh8")
    nc.vector.tensor_mul(hh8[:], h8[:], h8[:])
    alpha = sp.tile([128, 8], F32, tag="alpha")
    nc.vector.tensor_scalar(alpha[:], hh8[:], 2.0, 1.0, ALU.mult, ALU.subtract)

    # --- W[:, 3j:3j+3] = [1, alpha_j, beta_j]
    W = sp.tile([128, 24], F32, tag="W")
    nc.vector.memset(W[:], 1.0)
    for j in range(8):
        nc.vector.tensor_copy(W[:, 3 * j + 1 : 3 * j + 2], alpha[:, j : j + 1])
        nc.vector.tensor_copy(W[:, 3 * j + 2 : 3 * j + 3], beta[:, j : j + 1])

    # --- transposes -> [n-part, k-free] subtiles; S-side sums (contract n)
    psC = ps_cs.tile([128, 3], F32, tag="psC")
    psS = ps_cs.tile([128, 3], F32, tag="psS")
    for j in range(8):
        sl = slice(128 * j, 128 * (j + 1))
        tpc = ps_t.tile([128, 128], F32, tag="tp")
        nc.tensor.transpose(tpc[:], cmat[:, sl], ident[:])
        cnk = wp.tile([128, 128], F32, tag="cnk")
        nc.vector.tensor_copy(cnk[:], tpc[:])
        nc.tensor.matmul(psC[:], cnk[:], W[:, 3 * j : 3 * j + 3],
                         start=(j == 0), stop=(j == 7))
        tps = ps_t.tile([128, 128], F32, tag="tp")
        nc.tensor.transpose(tps[:], smat[:, sl], ident[:])
        snk = wp.tile([128, 128], F32, tag="snk")
        nc.vector.tensor_copy(snk[:], tps[:])
        nc.tensor.matmul(psS[:], snk[:], W[:, 3 * j : 3 * j + 3],
                         start=(j == 0), stop=(j == 7))

    # --- S_R, S_I  [128, 1]
    tmp1 = sp.tile([128, 1], F32, tag="tmp1")
    nc.vector.tensor_scalar(tmp1[:], psC[:, 1:2], cst[:, 1:2], None, ALU.mult)
    SR = sp.tile([128, 1], F32, tag="SR")
    nc.vector.scalar_tensor_tensor(SR[:], psC[:, 0:1], cst[:, 0:1], tmp1[:],
                                   ALU.mult, ALU.add)
    nc.vector.scalar_tensor_tensor(SR[:], psS[:, 2:3], cst[:, 2:3], SR[:],
                                   ALU.mult, ALU.add)
    tmp2 = sp.tile([128, 1], F32, tag="tmp2")
    nc.vector.tensor_scalar(tmp2[:], psS[:, 1:2], cst[:, 1:2], None, ALU.mult)
    SI = sp.tile([128, 1], F32, tag="SI")
    nc.vector.scalar_tensor_tensor(SI[:], psS[:, 0:1], cst[:, 0:1], tmp2[:],
                                   ALU.mult, ALU.add)
    nc.vector.scalar_tensor_tensor(SI[:], psC[:, 2:3], cst[:, 3:4], SI[:],
                                   ALU.mult, ALU.add)

    # --- U vectors [128, 6]; col order: [pw1*X, qw1*X, pw2*X, qw2*X, dw1*Y, dw2*Y]
    UC = sp.tile([128, 6], F32, tag="UC")
    US = sp.tile([128, 6], F32, tag="US")
    nc.vector.tensor_mul(UC[:, 0:4], cst[:, 4:8], _bc(SR, 4))
    nc.vector.tensor_mul(UC[:, 4:6], cst[:, 8:10], _bc(SI, 2))
    nc.vector.tensor_mul(US[:, 0:4], cst[:, 4:8], _bc(SI, 4))
    nc.vector.tensor_mul(US[:, 4:6], cst[:, 10:12], _bc(SR, 2))

    # --- T-side: out[n, 6] per subtile; regions of [128, 48] PSUMs
    pTC = ps_T.tile([128, 48], F32, tag="pTC")
    pTS = ps_T.tile([128, 48], F32, tag="pTS")
    for j in range(8):
        sl = slice(128 * j, 128 * (j + 1))
        nc.tensor.matmul(pTC[:, 6 * j : 6 * j + 6], cmat[:, sl], UC[:],
                         start=True, stop=True)
        nc.tensor.matmul(pTS[:, 6 * j : 6 * j + 6], smat[:, sl], US[:],
                         start=True, stop=True)

    # --- combine + self-energy + store
    # (TensorTensor may read at most one PSUM operand -> stage pTS in SBUF)
    sTS = sp.tile([128, 48], F32, tag="sTS")
    nc.vector.tensor_copy(sTS[:], pTS[:])
    aa = sp.tile([128, 8], F32, tag="aa")
    nc.vector.tensor_mul(aa[:], alpha[:], alpha[:])
    bb = sp.tile([128, 8], F32, tag="bb")
    nc.vector.tensor_mul(bb[:], beta[:], beta[:])

    for i, (cA, cB, cC, sc) in enumerate(((0, 1, 4, sc1), (2, 3, 5, sc2))):
        TT = sp.tile([128, 8], F32, tag="TT")
        nc.vector.tensor_add(TT[:], _st(pTC, cA), _st(sTS, cA))
        tb = sp.tile([128, 8], F32, tag="tb")
        nc.vector.tensor_add(tb[:], _st(pTC, cB), _st(sTS, cB))
        tb2 = sp.tile([128, 8], F32, tag="tb2")
        nc.vector.tensor_mul(tb2[:], tb[:], alpha[:])
        nc.vector.tensor_add(TT[:], TT[:], tb2[:])
        tcc = sp.tile([128, 8], F32, tag="tcc")
        nc.vector.tensor_add(tcc[:], _st(pTC, cC), _st(sTS, cC))
        tc2 = sp.tile([128, 8], F32, tag="tc2")
        nc.vector.tensor_mul(tc2[:], tcc[:], beta[:])
        nc.vector.tensor_add(TT[:], TT[:], tc2[:])

        sacc = sp.tile([128, 8], F32, tag="sacc")
        nc.vector.tensor_scalar(sacc[:], alpha[:], sc[1], sc[0], ALU.mult, ALU.add)
        nc.vector.scalar_tensor_tensor(sacc[:], aa[:], sc[2], sacc[:],
                                       ALU.mult, ALU.add)
        nc.vector.scalar_tensor_tensor(sacc[:], bb[:], sc[3], sacc[:],
                                       ALU.mult, ALU.add)
        ei = sp.tile([128, 8], F32, tag="ei")
        nc.vector.tensor_sub(ei[:], TT[:], sacc[:])
        nc.sync.dma_start(out_t[b].rearrange("(j p) c -> p j c", p=128)[:, :, i],
                          ei[:])


def _bc(col_ap, n):
    """Broadcast a [128, 1] tile AP along free dim to [128, n] (step 0)."""
    ap = col_ap[:]
    return bass.AP(ap.tensor, ap.offset, [ap.ap[0], [0, n]])


def _st(psum_tile, col):
    """Strided [128, 8] view of [128, 48] PSUM: cols col, col+6, ..."""
    ap = psum_tile[:]
    return bass.AP(ap.tensor, ap.offset + col, [ap.ap[0], [6, 8]])


def _build_program(sc1, sc2, debug=False):
    nc = bacc.Bacc(None, target_bir_lowering=False, debug=debug)
    t_in = nc.declare_dram_parameter("t", [BPC, N], F32, isOutput=False)
    kv_in = nc.declare_dram_parameter("kv", [1, KTRUNC], F32, isOutput=False)
    cst_in = nc.declare_dram_parameter("cst", [128, 12], F32, isOutput=False)
    id_in = nc.declare_dram_parameter("ident", [128, 128], F32, isOutput=False)
    out_t = nc.declare_dram_parameter("out", [BPC, N, 2], F32, isOutput=True)

    with tile.TileContext(nc) as tc:
        import contextlib
        with contextlib.ExitStack() as ctx:
            pc = ctx.enter_context(tc.tile_pool(name="const", bufs=1))
            wp = ctx.enter_context(tc.tile_pool(name="work", bufs=2))
            sp = ctx.enter_context(tc.tile_pool(name="small", bufs=2))
            ps_u = ctx.enter_context(tc.tile_pool(name="psu", bufs=1, space="PSUM"))
            ps_t = ctx.enter_context(tc.tile_pool(name="pst", bufs=2, space="PSUM"))
            ps_cs = ctx.enter_context(tc.tile_pool(name="pscs", bufs=1, space="PSUM"))
            ps_T = ctx.enter_context(tc.tile_pool(name="psT", bufs=1, space="PSUM"))

            ident = pc.tile([128, 128], F32, tag="ident")
            nc.sync.dma_start(ident[:], id_in[:])
            cst = pc.tile([128, 12], F32, tag="cst")
            nc.sync.dma_start(cst[:], cst_in[:])
            kv = pc.tile([1, KTRUNC], F32, tag="kv")
            nc.sync.dma_start(kv[:], kv_in[:])

            pools = (pc, wp, sp, ps_u, ps_t, ps_cs, ps_T)
            for b in range(BPC):
                _emit_batch(nc, tc, pools, b, t_in, out_t, kv, cst, ident,
                            sc1, sc2)
    return nc


def kernel(x, shift0, shift1, amp0, amp1):
    x = np.asarray(x, dtype=np.float32)
    cst, sc1, sc2 = _host_constants(shift0.reshape(-1)[0], shift1.reshape(-1)[0],
                                    amp0.reshape(-1)[0], amp1.reshape(-1)[0])
    nc = _build_program(sc1, sc2)
    nc.finalize()

    t_full = (x.astype(np.float64) / (2.0 * np.pi)).astype(np.float32)
    kvals = np.arange(KTRUNC, dtype=np.float32).reshape(1, KTRUNC)
    ident = np.eye(128, dtype=np.float32)
    in_maps = []
    for c in range(NCORES):
        in_maps.append({
            "t": t_full[BPC * c : BPC * (c + 1)],
            "kv": kvals,
            "cst": cst,
            "ident": ident,
        })
    res = run_bass_kernel_spmd(nc, in_maps, list(range(NCORES)))
    out = np.concatenate([res.results[c]["out"] for c in range(NCORES)], axis=0)
    return out.astype(np.float32)



# revision 7
# speedup vs baseline: 1.0194x; 1.0194x over previous
"""Trainium2 Bass kernel for nn_NUFFTLayerMultiChannelInitMixed.

Math: the reference's spread->FFT->filter->IFFT->energy pipeline is an exact
bilinear form in the analytic spectrum of the periodized Gaussians.  With the
M-aliased images dropped (their weight is exp(-tau*(k-M)^2) ~ 3e-5) the
energy reduces to a truncated cosine series in the K lowest modes:

  e_i[n] = sum_{k<K} g_ik * ( cs_k cos(k x_n) + ss_k sin(k x_n) ) - self_i
  cs_k   = sum_n cos(k x_n),   ss_k = sum_n sin(k x_n)
  g_ik   = pref * w_k * deconv^2_k * mult_ik * p_k^2   (host precomputed)

K=64 keeps rel err ~3e-4.  Each core packs its BPC=2 batches into the 128
partitions as [batch0: k=0..63 | batch1: k=0..63] with block-diagonal
stationaries, so the whole core workload is ONE [128, 1024] problem:

  phases  u = kst^T @ t3 on PE (fp16 3-term split of t keeps u exact to 2e-6)
  range reduction via the fp32 magic-rounding trick (2 DVE ops)
  sin/cos via two Sin activations (+ 1-2sin^2), accum_out gives cs/ss free
  e = a_blk^T @ cos + b_blk^T @ sin   (fp16, two 512-col PSUM banks)

self_i folds into the k=0 coefficient (cos(0)=1 row), so the PSUM result is
final and DMAs straight to DRAM.  No transposes, no collectives.
"""

import numpy as np

try:
    import concourse.bass as bass
except ImportError:
    import sys
    sys.path.insert(0, "/opt/trn_rl_repo")
    import concourse.bass as bass

import concourse.bacc as bacc
import concourse.mybir as mybir
from concourse import tile
from concourse.bass_utils import run_bass_kernel_spmd

F32 = mybir.dt.float32
F16 = mybir.dt.float16
AF = mybir.ActivationFunctionType
ALU = mybir.AluOpType

M = 2001
L = 2.0 * np.pi
TAU = 12.0 * (L / (2.0 * np.pi * M)) ** 2
K = 64                  # modes kept per batch
B_FULL, N = 16, 1024
NCORES = 8
BPC = B_FULL // NCORES  # 2 batches per core, packed into 2*K=128 partitions
MAGIC = 12582912.0      # 1.5 * 2^23: fl(u + MAGIC) - MAGIC = round-to-nearest(u)
PI = float(np.pi)


def _host_tables(shift0, shift1, amp0, amp1):
    """fp64 k-space tables: block-diag gain gH [128,4], self-energy HH [128,4],
    phase stationary kst [6,128] fp16."""
    k = np.arange(K, dtype=np.float64)
    p = np.exp(-TAU * k * k)
    Cc = (M / L) * np.sqrt(4.0 * np.pi * TAU)
    deconv2 = (np.pi / TAU) * np.exp(2.0 * TAU * k * k)
    mult1 = float(amp0) * 4.0 * np.pi / (k * k + (1.0 * float(shift0)) ** 2)
    mult2 = float(amp1) * 4.0 * np.pi / (k * k + (0.5 * float(shift1)) ** 2)
    w = np.full(K, 2.0)
    w[0] = 1.0
    scale = 1.0 / ((2.0 * np.pi * M / L) * (2.0 * np.pi))
    pref = scale * Cc * Cc / M
    g = np.stack([pref * w * deconv2 * mult1 * p * p,
                  pref * w * deconv2 * mult2 * p * p], axis=1)   # [K, 2]
    self2 = g.sum(axis=0)                                        # [2]

    gH = np.zeros((128, 4), np.float32)
    gH[0:K, 0:2] = g
    gH[K:128, 2:4] = g
    HH = np.zeros((128, 4), np.float32)
    HH[0, 0:2] = -self2
    HH[K, 2:4] = -self2
    kst = np.zeros((6, 128), np.float32)
    kst[0:3, 0:K] = k[None, :]
    kst[3:6, K:128] = k[None, :]
    return gH, HH, kst.astype(np.float16)


def _split3(t):
    """t fp64 -> three fp16 arrays with t0+t1+t2 == t to ~2^-25."""
    t0 = t.astype(np.float16)
    r = t - t0.astype(np.float64)
    t1 = r.astype(np.float16)
    r = r - t1.astype(np.float64)
    t2 = r.astype(np.float16)
    return t0, t1, t2


def _build_program(debug=False):
    nc = bacc.Bacc(None, target_bir_lowering=False, debug=debug)
    t3_in = nc.declare_dram_parameter("t3", [6, N], F16, isOutput=False)
    kst_in = nc.declare_dram_parameter("kst", [6, 128], F16, isOutput=False)
    gH_in = nc.declare_dram_parameter("gH", [128, 4], F32, isOutput=False)
    HH_in = nc.declare_dram_parameter("HH", [128, 4], F32, isOutput=False)
    out_t = nc.declare_dram_parameter("out", [BPC, N, 2], F32, isOutput=True)

    with tile.TileContext(nc) as tc:
        import contextlib
        with contextlib.ExitStack() as ctx:
            pc = ctx.enter_context(tc.tile_pool(name="const", bufs=1))
            wp = ctx.enter_context(tc.tile_pool(name="work", bufs=1))
            ps_u = ctx.enter_context(tc.tile_pool(name="psu", bufs=1, space="PSUM"))
            ps_e = ctx.enter_context(tc.tile_pool(name="pse", bufs=1, space="PSUM"))

            kst = pc.tile([6, 128], F16, tag="kst")
            nc.sync.dma_start(kst[:], kst_in[:])
            t3 = pc.tile([6, N], F16, tag="t3")
            nc.sync.dma_start(t3[:], t3_in[:])
            gH = pc.tile([128, 4], F32, tag="gH")
            nc.sync.dma_start(gH[:], gH_in[:])
            HH = pc.tile([128, 4], F32, tag="HH")
            nc.sync.dma_start(HH[:], HH_in[:])

            # phases u[j, n] = k_j * t_{b(j), n}  (j<64: batch0, j>=64: batch1)
            u = ps_u.tile([128, N], F32, tag="u")
            nc.tensor.matmul(u[:, 0:512], kst[:], t3[:, 0:512], start=True, stop=True)
            nc.tensor.matmul(u[:, 512:1024], kst[:], t3[:, 512:1024], start=True, stop=True)

            # negr = round(u) - u  (exact); sin(-2*pi*negr) = sin(2*pi*u)
            v = wp.tile([128, N], F32, tag="v")
            nc.vector.tensor_scalar(v[:], u[:], MAGIC, None, ALU.add)
            negr = wp.tile([128, N], F32, tag="negr")
            nc.vector.scalar_tensor_tensor(negr[:], v[:], MAGIC, u[:],
                                           ALU.subtract, ALU.subtract)

            h = wp.tile([128, N], F32, tag="h")
            nc.scalar.activation(h[:], negr[:], AF.Sin, scale=PI)
            hh2 = wp.tile([128, N], F32, tag="hh2")
            nc.scalar.activation(hh2[:], h[:], AF.Square,
                                 scale=float(np.sqrt(2.0)))
            ss = wp.tile([128, 1], F32, tag="ss")
            smat = wp.tile([128, N], F16, tag="smat")
            nc.scalar.activation(smat[:], negr[:], AF.Sin, scale=-2.0 * PI,
                                 accum_out=ss[:])
            # mneg = -2 sin^2(pi r) = cos(2 pi r) - 1
            # tensor_scalar w/ accum_out: out = in0 op0 s1; accum = reduce(out, op1, init=s2)
            mneg = wp.tile([128, N], F32, tag="mneg")
            cs = wp.tile([128, 1], F32, tag="cs")
            nc.vector.tensor_scalar(mneg[:], hh2[:], -1.0, 1024.0, ALU.mult,
                                    ALU.add, accum_out=cs[:])
            cmat = wp.tile([128, N], F16, tag="cmat")
            nc.vector.tensor_scalar(cmat[:], mneg[:], 1.0, None, ALU.add)

            # per-mode coefficients; self-energy folded into k=0 rows via HH
            a_blk = wp.tile([128, 4], F16, tag="a_blk")
            nc.vector.scalar_tensor_tensor(a_blk[:], gH[:], cs[:], HH[:],
                                           ALU.mult, ALU.add)
            b_blk = wp.tile([128, 4], F16, tag="b_blk")
            nc.vector.tensor_scalar(b_blk[:], gH[:], ss[:], None, ALU.mult)

            # e[(b c), n] = a^T cos + b^T sin
            e = ps_e.tile([4, N], F32, tag="e")
            nc.tensor.matmul(e[:, 0:512], a_blk[:], cmat[:, 0:512], start=True, stop=False)
            nc.tensor.matmul(e[:, 0:512], b_blk[:], smat[:, 0:512], start=False, stop=True)
            nc.tensor.matmul(e[:, 512:1024], a_blk[:], cmat[:, 512:1024], start=True, stop=False)
            nc.tensor.matmul(e[:, 512:1024], b_blk[:], smat[:, 512:1024], start=False, stop=True)

            es = wp.tile([4, N], F32, tag="es")
            nc.scalar.activation(es[:, 0:512], e[:, 0:512], AF.Copy)
            nc.vector.tensor_copy(es[:, 512:1024], e[:, 512:1024])
            for b in range(BPC):
                nc.sync.dma_start(out_t[b].rearrange("n c -> c n"),
                                  es[2 * b : 2 * b + 2, :])
    return nc


def _make_in_maps(x, shift0, shift1, amp0, amp1):
    gH, HH, kst = _host_tables(shift0.reshape(-1)[0], shift1.reshape(-1)[0],
                               amp0.reshape(-1)[0], amp1.reshape(-1)[0])
    t = np.asarray(x, np.float64) / (2.0 * np.pi)
    t0, t1, t2 = _split3(t)
    in_maps = []
    for c in range(NCORES):
        b0, b1 = BPC * c, BPC * c + 1
        t3 = np.stack([t0[b0], t1[b0], t2[b0], t0[b1], t1[b1], t2[b1]])
        in_maps.append({"t3": t3, "kst": kst, "gH": gH, "HH": HH})
    return in_maps


def kernel(x, shift0, shift1, amp0, amp1):
    in_maps = _make_in_maps(x, shift0, shift1, amp0, amp1)
    nc = _build_program()
    nc.finalize()
    res = run_bass_kernel_spmd(nc, in_maps, list(range(NCORES)))
    out = np.concatenate([res.results[c]["out"] for c in range(NCORES)], axis=0)
    return out.astype(np.float32)


# revision 11
# speedup vs baseline: 3.0475x; 2.9895x over previous
"""Trainium2 Bass kernel for nn_NUFFTLayerMultiChannelInitMixed.

Math: the reference's spread->FFT->filter->IFFT->energy pipeline is an exact
bilinear form in the analytic spectrum of the periodized Gaussians.  With the
M-aliased images dropped (their weight is exp(-tau*(k-M)^2) ~ 3e-5) the
energy reduces to a truncated cosine series in the K lowest modes:

  e_i[n] = sum_{k<K} g_ik * ( cs_k cos(k x_n) + ss_k sin(k x_n) ) - self_i
  cs_k   = sum_n cos(k x_n),   ss_k = sum_n sin(k x_n)
  g_ik   = pref * w_k * deconv^2_k * mult_ik * p_k^2   (host precomputed)

K=64 keeps rel err ~3e-4.  Each core packs its BPC=2 batches into the 128
partitions as [batch0: k=0..63 | batch1: k=0..63] with block-diagonal
stationaries, so the whole core workload is ONE [128, 1024] problem:

  phases  u = kst^T @ t3 on PE (fp16 3-term split of t keeps u exact to 2e-6)
  range reduction via the fp32 magic-rounding trick (2 DVE ops)
  sin/cos via two Sin activations (+ 1-2sin^2), accum_out gives cs/ss free
  e = a_blk^T @ cos + b_blk^T @ sin   (fp16, two 512-col PSUM banks)

self_i folds into the k=0 coefficient (cos(0)=1 row), so the PSUM result is
final and DMAs straight to DRAM.  No transposes, no collectives.
"""

import numpy as np

try:
    import concourse.bass as bass
except ImportError:
    import sys
    sys.path.insert(0, "/opt/trn_rl_repo")
    import concourse.bass as bass

import concourse.bacc as bacc
import concourse.mybir as mybir
from concourse import tile
from concourse.bass_utils import run_bass_kernel_spmd

F32 = mybir.dt.float32
F16 = mybir.dt.float16
AF = mybir.ActivationFunctionType
ALU = mybir.AluOpType

M = 2001
L = 2.0 * np.pi
TAU = 12.0 * (L / (2.0 * np.pi * M)) ** 2
K = 64                  # modes kept per batch
B_FULL, N = 16, 1024
NCORES = 8
BPC = B_FULL // NCORES  # 2 batches per core, packed into 2*K=128 partitions
MAGIC = 12582912.0      # 1.5 * 2^23: fl(u + MAGIC) - MAGIC = round-to-nearest(u)
PI = float(np.pi)


def _host_tables(shift0, shift1, amp0, amp1):
    """fp64 k-space tables: block-diag gain gH [128,4], self-energy HH [128,4],
    phase stationary kst [6,128] fp16."""
    k = np.arange(K, dtype=np.float64)
    p = np.exp(-TAU * k * k)
    Cc = (M / L) * np.sqrt(4.0 * np.pi * TAU)
    deconv2 = (np.pi / TAU) * np.exp(2.0 * TAU * k * k)
    mult1 = float(amp0) * 4.0 * np.pi / (k * k + (1.0 * float(shift0)) ** 2)
    mult2 = float(amp1) * 4.0 * np.pi / (k * k + (0.5 * float(shift1)) ** 2)
    w = np.full(K, 2.0)
    w[0] = 1.0
    scale = 1.0 / ((2.0 * np.pi * M / L) * (2.0 * np.pi))
    pref = scale * Cc * Cc / M
    g = np.stack([pref * w * deconv2 * mult1 * p * p,
                  pref * w * deconv2 * mult2 * p * p], axis=1)   # [K, 2]
    self2 = g.sum(axis=0)                                        # [2]

    gH = np.zeros((128, 4), np.float32)
    gH[0:K, 0:2] = g
    gH[K:128, 2:4] = g
    HH = np.zeros((128, 4), np.float32)
    HH[0, 0:2] = -self2
    HH[K, 2:4] = -self2
    kst = np.zeros((6, 128), np.float32)
    kst[0:3, 0:K] = k[None, :]
    kst[3:6, K:128] = k[None, :]
    return gH, HH, kst.astype(np.float16)


def _split3(t):
    """t fp64 -> three fp16 arrays with t0+t1+t2 == t to ~2^-25."""
    t0 = t.astype(np.float16)
    r = t - t0.astype(np.float64)
    t1 = r.astype(np.float16)
    r = r - t1.astype(np.float64)
    t2 = r.astype(np.float16)
    return t0, t1, t2


def _build_program(debug=False):
    nc = bacc.Bacc(None, target_bir_lowering=False, debug=debug)
    # kt3 = [kst | t3] packed fp16; gHH = [gH | HH] packed f32
    kt3_in = nc.declare_dram_parameter("kt3", [6, 128 + N], F16, isOutput=False)
    gHH_in = nc.declare_dram_parameter("gHH", [128, 8], F32, isOutput=False)
    out_t = nc.declare_dram_parameter("out", [BPC, 2, N], F32, isOutput=True)

    with tile.TileContext(nc) as tc:
        import contextlib
        with contextlib.ExitStack() as ctx:
            pc = ctx.enter_context(tc.tile_pool(name="const", bufs=1))
            wp = ctx.enter_context(tc.tile_pool(name="work", bufs=1))
            ps_u = ctx.enter_context(tc.tile_pool(name="psu", bufs=1, space="PSUM"))
            ps_e = ctx.enter_context(tc.tile_pool(name="pse", bufs=1, space="PSUM"))

            kt3 = pc.tile([6, 128 + N], F16, tag="kt3")
            nc.sync.dma_start(kt3[:], kt3_in[:])
            gHH = pc.tile([128, 8], F32, tag="gHH")
            nc.sync.dma_start(gHH[:], gHH_in[:])
            kst = kt3[:, 0:128]
            t3 = kt3[:, 128 : 128 + N]
            gH = gHH[:, 0:4]
            HH = gHH[:, 4:8]

            # phases u[j, n] = k_j * t_{b(j), n}  (j<64: batch0, j>=64: batch1)
            u = ps_u.tile([128, N], F32, tag="u")
            nc.tensor.matmul(u[:, 0:512], kst[:], t3[:, 0:512], start=True, stop=True)
            nc.tensor.matmul(u[:, 512:1024], kst[:], t3[:, 512:1024], start=True, stop=True)

            # negr = round(u) - u  (exact); sin(-2*pi*negr) = sin(2*pi*u)
            v = wp.tile([128, N], F32, tag="v")
            nc.vector.tensor_scalar(v[:], u[:], MAGIC, None, ALU.add)
            negr = wp.tile([128, N], F32, tag="negr")
            nc.vector.scalar_tensor_tensor(negr[:], v[:], MAGIC, u[:],
                                           ALU.subtract, ALU.subtract)

            h = wp.tile([128, N], F32, tag="h")
            nc.scalar.activation(h[:], negr[:], AF.Sin, scale=PI)
            hh2 = wp.tile([128, N], F32, tag="hh2")
            nc.scalar.activation(hh2[:], h[:], AF.Square,
                                 scale=float(np.sqrt(2.0)))
            ss = wp.tile([128, 1], F32, tag="ss")
            smat = wp.tile([128, N], F16, tag="smat")
            nc.scalar.activation(smat[:], negr[:], AF.Sin, scale=-2.0 * PI,
                                 accum_out=ss[:])
            # mneg = -2 sin^2(pi r) = cos(2 pi r) - 1
            # tensor_scalar w/ accum_out: out = in0 op0 s1; accum = reduce(out, op1, init=s2)
            mneg = wp.tile([128, N], F32, tag="mneg")
            cs = wp.tile([128, 1], F32, tag="cs")
            nc.vector.tensor_scalar(mneg[:], hh2[:], -1.0, 1024.0, ALU.mult,
                                    ALU.add, accum_out=cs[:])
            cmat = wp.tile([128, N], F16, tag="cmat")
            nc.vector.tensor_scalar(cmat[:], mneg[:], 1.0, None, ALU.add)

            # per-mode coefficients; self-energy folded into k=0 rows via HH
            a_blk = wp.tile([128, 4], F16, tag="a_blk")
            nc.vector.scalar_tensor_tensor(a_blk[:], gH[:], cs[:], HH[:],
                                           ALU.mult, ALU.add)
            b_blk = wp.tile([128, 4], F16, tag="b_blk")
            nc.vector.tensor_scalar(b_blk[:], gH[:], ss[:], None, ALU.mult)

            # e[(b c), n] = a^T cos + b^T sin
            e = ps_e.tile([4, N], F32, tag="e")
            nc.tensor.matmul(e[:, 0:512], a_blk[:], cmat[:, 0:512], start=True, stop=False)
            nc.tensor.matmul(e[:, 0:512], b_blk[:], smat[:, 0:512], start=False, stop=True)
            nc.tensor.matmul(e[:, 512:1024], a_blk[:], cmat[:, 512:1024], start=True, stop=False)
            nc.tensor.matmul(e[:, 512:1024], b_blk[:], smat[:, 512:1024], start=False, stop=True)

            es = wp.tile([4, N], F32, tag="es")
            nc.scalar.activation(es[:, 0:512], e[:, 0:512], AF.Copy)
            nc.vector.tensor_copy(es[:, 512:1024], e[:, 512:1024])
            # out rows (b0c0, b0c1, b1c0, b1c1) -> out[b, c, n] contiguous
            nc.sync.dma_start(out_t.rearrange("b c n -> (b c) n"), es[:])
    return nc


def _make_in_maps(x, shift0, shift1, amp0, amp1):
    gH, HH, kst = _host_tables(shift0.reshape(-1)[0], shift1.reshape(-1)[0],
                               amp0.reshape(-1)[0], amp1.reshape(-1)[0])
    gHH = np.concatenate([gH, HH], axis=1)                      # [128, 8]
    t = np.asarray(x, np.float64) / (2.0 * np.pi)
    t0, t1, t2 = _split3(t)
    in_maps = []
    for c in range(NCORES):
        b0, b1 = BPC * c, BPC * c + 1
        t3 = np.stack([t0[b0], t1[b0], t2[b0], t0[b1], t1[b1], t2[b1]])
        kt3 = np.concatenate([kst, t3], axis=1)                 # [6, 1152]
        in_maps.append({"kt3": kt3, "gHH": gHH})
    return in_maps


def kernel(x, shift0, shift1, amp0, amp1):
    in_maps = _make_in_maps(x, shift0, shift1, amp0, amp1)
    nc = _build_program()
    nc.finalize()
    res = run_bass_kernel_spmd(nc, in_maps, list(range(NCORES)))
    # device emits [BPC, 2, N]; reference wants [B, N, 2]
    out = np.concatenate([res.results[c]["out"] for c in range(NCORES)], axis=0)
    return np.ascontiguousarray(out.transpose(0, 2, 1)).astype(np.float32)


# revision 17
# speedup vs baseline: 3.3602x; 1.1026x over previous
"""Trainium2 Bass kernel for nn_NUFFTLayerMultiChannelInitMixed.

Math: the reference's spread->FFT->filter->IFFT->energy pipeline is an exact
bilinear form in the analytic spectrum of the periodized Gaussians.  With the
M-aliased images dropped (their weight is exp(-tau*(k-M)^2) ~ 3e-5) the
energy reduces to a truncated cosine series in the K lowest modes:

  e_i[n] = sum_{k<K} g_ik * ( cs_k cos(k x_n) + ss_k sin(k x_n) ) - self_i
  cs_k   = sum_n cos(k x_n),   ss_k = sum_n sin(k x_n)
  g_ik   = pref * w_k * deconv^2_k * mult_ik * p_k^2   (host precomputed)

K=64 keeps rel err ~3e-4.  Each core packs its BPC=2 batches into the 128
partitions as [batch0: k=0..63 | batch1: k=0..63] with block-diagonal
stationaries, so the whole core workload is ONE [128, 1024] problem:

  phases  u = kst^T @ t3 on PE (fp16 3-term split of t keeps u exact to 2e-6)
  range reduction via the fp32 magic-rounding trick (2 DVE ops)
  sin/cos via two Sin activations (+ 1-2sin^2), accum_out gives cs/ss free
  e = a_blk^T @ cos + b_blk^T @ sin   (fp16, two 512-col PSUM banks)

self_i folds into the k=0 coefficient (cos(0)=1 row), so the PSUM result is
final and DMAs straight to DRAM.  No transposes, no collectives.
"""

import numpy as np

try:
    import concourse.bass as bass
except ImportError:
    import sys
    sys.path.insert(0, "/opt/trn_rl_repo")
    import concourse.bass as bass

import concourse.bacc as bacc
import concourse.mybir as mybir
from concourse import tile
from concourse.bass_utils import run_bass_kernel_spmd

F32 = mybir.dt.float32
F16 = mybir.dt.float16
AF = mybir.ActivationFunctionType
ALU = mybir.AluOpType

M = 2001
L = 2.0 * np.pi
TAU = 12.0 * (L / (2.0 * np.pi * M)) ** 2
K = 64                  # modes kept per batch
B_FULL, N = 16, 1024
NCORES = 8
BPC = B_FULL // NCORES  # 2 batches per core, packed into 2*K=128 partitions
MAGIC = 12582912.0      # 1.5 * 2^23: fl(u + MAGIC) - MAGIC = round-to-nearest(u)
PI = float(np.pi)


def _host_tables(shift0, shift1, amp0, amp1):
    """fp64 k-space tables: block-diag gain gH [128,4], self-energy HH [128,4],
    phase stationary kst [6,128] fp16."""
    k = np.arange(K, dtype=np.float64)
    p = np.exp(-TAU * k * k)
    Cc = (M / L) * np.sqrt(4.0 * np.pi * TAU)
    deconv2 = (np.pi / TAU) * np.exp(2.0 * TAU * k * k)
    mult1 = float(amp0) * 4.0 * np.pi / (k * k + (1.0 * float(shift0)) ** 2)
    mult2 = float(amp1) * 4.0 * np.pi / (k * k + (0.5 * float(shift1)) ** 2)
    w = np.full(K, 2.0)
    w[0] = 1.0
    scale = 1.0 / ((2.0 * np.pi * M / L) * (2.0 * np.pi))
    pref = scale * Cc * Cc / M
    g = np.stack([pref * w * deconv2 * mult1 * p * p,
                  pref * w * deconv2 * mult2 * p * p], axis=1)   # [K, 2]
    self2 = g.sum(axis=0)                                        # [2]

    gH = np.zeros((128, 4), np.float32)
    gH[0:K, 0:2] = g
    gH[K:128, 2:4] = g
    # a_blk = gH*csneg + HH2 with csneg = cs - 1024  =>  HH2 = 1024*gH + HH
    HH = np.zeros((128, 4), np.float32)
    HH[0, 0:2] = -self2
    HH[K, 2:4] = -self2
    HH = (1024.0 * gH.astype(np.float64) + HH).astype(np.float32)
    kst = np.zeros((6, 128), np.float32)
    kst[0:3, 0:K] = k[None, :]
    kst[3:6, K:128] = k[None, :]
    return gH, HH, kst.astype(np.float16)


def _split3(t):
    """t fp64 -> three fp16 arrays with t0+t1+t2 == t to ~2^-25."""
    t0 = t.astype(np.float16)
    r = t - t0.astype(np.float64)
    t1 = r.astype(np.float16)
    r = r - t1.astype(np.float64)
    t2 = r.astype(np.float16)
    return t0, t1, t2


def _build_program(debug=False):
    nc = bacc.Bacc(None, target_bir_lowering=False, debug=debug)
    # kt3 = [kst | t3] packed fp16; gHH = [gH | HH] packed f32
    kt3_in = nc.declare_dram_parameter("kt3", [6, 128 + N], F16, isOutput=False)
    gHH_in = nc.declare_dram_parameter("gHH", [128, 8], F32, isOutput=False)
    out_t = nc.declare_dram_parameter("out", [BPC, 2, N], F32, isOutput=True)

    with tile.TileContext(nc) as tc:
        import contextlib
        with contextlib.ExitStack() as ctx:
            pc = ctx.enter_context(tc.tile_pool(name="const", bufs=1))
            wp = ctx.enter_context(tc.tile_pool(name="work", bufs=1))
            ps_u = ctx.enter_context(tc.tile_pool(name="psu", bufs=1, space="PSUM"))
            ps_e = ctx.enter_context(tc.tile_pool(name="pse", bufs=1, space="PSUM"))

            kt3 = pc.tile([6, 128 + N], F16, tag="kt3")
            nc.gpsimd.dma_start(kt3[:], kt3_in[:])
            gHH = pc.tile([128, 8], F32, tag="gHH")
            nc.scalar.dma_start(gHH[:], gHH_in[:])
            kst = kt3[:, 0:128]
            t3 = kt3[:, 128 : 128 + N]
            gH = gHH[:, 0:4]
            HH = gHH[:, 4:8]

            # phases u[j, n] = k_j * t_{b(j), n}  (j<64: batch0, j>=64: batch1)
            u = ps_u.tile([128, N], F32, tag="u")
            nc.tensor.matmul(u[:, 0:512], kst[:], t3[:, 0:512], start=True, stop=True)
            nc.tensor.matmul(u[:, 512:1024], kst[:], t3[:, 512:1024], start=True, stop=True)

            # negr = round(u) - u  (exact); sin(-2*pi*negr) = sin(2*pi*u)
            v = wp.tile([128, N], F32, tag="v")
            nc.vector.tensor_scalar(v[:], u[:], MAGIC, None, ALU.add)
            negr = wp.tile([128, N], F32, tag="negr")
            nc.vector.scalar_tensor_tensor(negr[:], v[:], MAGIC, u[:],
                                           ALU.subtract, ALU.subtract)

            h = wp.tile([128, N], F32, tag="h")
            nc.scalar.activation(h[:], negr[:], AF.Sin, scale=PI)
            ss = wp.tile([128, 1], F32, tag="ss")
            smat = wp.tile([128, N], F16, tag="smat")
            nc.scalar.activation(smat[:], negr[:], AF.Sin, scale=-2.0 * PI,
                                 accum_out=ss[:])
            # sin branch ready first: b_blk + its matmuls overlap the cos chain
            b_blk = wp.tile([128, 4], F16, tag="b_blk")
            nc.vector.tensor_scalar(b_blk[:], gH, ss[:], None, ALU.mult)

            # mneg = -2 sin^2(pi r) = cos(2 pi r) - 1;  csneg = sum(mneg) = cs - 1024
            mneg = wp.tile([128, N], F32, tag="mneg")
            csneg = wp.tile([128, 1], F32, tag="csneg")
            nc.vector.scalar_tensor_tensor(mneg[:], h[:], -2.0, h[:],
                                           ALU.mult, ALU.mult,
                                           accum_out=csneg[:])
            cmat = wp.tile([128, N], F16, tag="cmat")
            nc.vector.tensor_scalar(cmat[:], mneg[:], 1.0, None, ALU.add)

            # a_blk = gH*csneg + HH2 (self-energy + 1024*gH folded host-side)
            a_blk = wp.tile([128, 4], F16, tag="a_blk")
            nc.vector.scalar_tensor_tensor(a_blk[:], gH, csneg[:], HH,
                                           ALU.mult, ALU.add)

            # e[(b c), n] = a^T cos + b^T sin; sin first (ready earlier)
            e = ps_e.tile([4, N], F32, tag="e")
            nc.tensor.matmul(e[:, 0:512], b_blk[:], smat[:, 0:512], start=True, stop=False)
            nc.tensor.matmul(e[:, 512:1024], b_blk[:], smat[:, 512:1024], start=True, stop=False)
            nc.tensor.matmul(e[:, 0:512], a_blk[:], cmat[:, 0:512], start=False, stop=True)
            nc.tensor.matmul(e[:, 512:1024], a_blk[:], cmat[:, 512:1024], start=False, stop=True)

            es = wp.tile([4, N], F32, tag="es")
            nc.scalar.activation(es[:, 0:512], e[:, 0:512], AF.Copy)
            nc.vector.tensor_copy(es[:, 512:1024], e[:, 512:1024])
            # out rows (b0c0, b0c1, b1c0, b1c1) -> out[b, c, n] contiguous
            dst = out_t.rearrange("b c n -> (b c) n")
            nc.scalar.dma_start(dst[:, 0:512], es[:, 0:512])
            nc.gpsimd.dma_start(dst[:, 512:1024], es[:, 512:1024])
    return nc


def _make_in_maps(x, shift0, shift1, amp0, amp1):
    gH, HH, kst = _host_tables(shift0.reshape(-1)[0], shift1.reshape(-1)[0],
                               amp0.reshape(-1)[0], amp1.reshape(-1)[0])
    gHH = np.concatenate([gH, HH], axis=1)                      # [128, 8]
    t = np.asarray(x, np.float64) / (2.0 * np.pi)
    t0, t1, t2 = _split3(t)
    in_maps = []
    for c in range(NCORES):
        b0, b1 = BPC * c, BPC * c + 1
        t3 = np.stack([t0[b0], t1[b0], t2[b0], t0[b1], t1[b1], t2[b1]])
        kt3 = np.concatenate([kst, t3], axis=1)                 # [6, 1152]
        in_maps.append({"kt3": kt3, "gHH": gHH})
    return in_maps


def kernel(x, shift0, shift1, amp0, amp1):
    in_maps = _make_in_maps(x, shift0, shift1, amp0, amp1)
    nc = _build_program()
    nc.finalize()
    res = run_bass_kernel_spmd(nc, in_maps, list(range(NCORES)))
    # device emits [BPC, 2, N]; reference wants [B, N, 2]
    out = np.concatenate([res.results[c]["out"] for c in range(NCORES)], axis=0)
    return np.ascontiguousarray(out.transpose(0, 2, 1)).astype(np.float32)


# revision 19
# speedup vs baseline: 3.3738x; 1.0040x over previous
"""Trainium2 Bass kernel for nn_NUFFTLayerMultiChannelInitMixed.

Math: the reference's spread->FFT->filter->IFFT->energy pipeline is an exact
bilinear form in the analytic spectrum of the periodized Gaussians.  With the
M-aliased images dropped (their weight is exp(-tau*(k-M)^2) ~ 3e-5) the
energy reduces to a truncated cosine series in the K lowest modes:

  e_i[n] = sum_{k<K} g_ik * ( cs_k cos(k x_n) + ss_k sin(k x_n) ) - self_i
  cs_k   = sum_n cos(k x_n),   ss_k = sum_n sin(k x_n)
  g_ik   = pref * w_k * deconv^2_k * mult_ik * p_k^2   (host precomputed)

K=64 keeps rel err ~3e-4.  Each core packs its BPC=2 batches into the 128
partitions as [batch0: k=0..63 | batch1: k=0..63] with block-diagonal
stationaries, so the whole core workload is ONE [128, 1024] problem:

  phases  u = kst^T @ t3 on PE (fp16 3-term split of t keeps u exact to 2e-6)
  range reduction via the fp32 magic-rounding trick (2 DVE ops)
  sin/cos via two Sin activations (+ 1-2sin^2), accum_out gives cs/ss free
  e = a_blk^T @ cos + b_blk^T @ sin   (fp16, two 512-col PSUM banks)

self_i folds into the k=0 coefficient (cos(0)=1 row), so the PSUM result is
final and DMAs straight to DRAM.  No transposes, no collectives.
"""

import numpy as np

try:
    import concourse.bass as bass
except ImportError:
    import sys
    sys.path.insert(0, "/opt/trn_rl_repo")
    import concourse.bass as bass

import concourse.bacc as bacc
import concourse.mybir as mybir
from concourse import tile
from concourse.bass_utils import run_bass_kernel_spmd

F32 = mybir.dt.float32
F16 = mybir.dt.float16
AF = mybir.ActivationFunctionType
ALU = mybir.AluOpType

M = 2001
L = 2.0 * np.pi
TAU = 12.0 * (L / (2.0 * np.pi * M)) ** 2
K = 64                  # modes kept per batch
B_FULL, N = 16, 1024
NCORES = 8
BPC = B_FULL // NCORES  # 2 batches per core, packed into 2*K=128 partitions
MAGIC = 12582912.0      # 1.5 * 2^23: fl(u + MAGIC) - MAGIC = round-to-nearest(u)
PI = float(np.pi)


def _host_tables(shift0, shift1, amp0, amp1):
    """fp64 k-space tables: block-diag gain gH [128,4], self-energy HH [128,4],
    phase stationary kst [6,128] fp16."""
    k = np.arange(K, dtype=np.float64)
    p = np.exp(-TAU * k * k)
    Cc = (M / L) * np.sqrt(4.0 * np.pi * TAU)
    deconv2 = (np.pi / TAU) * np.exp(2.0 * TAU * k * k)
    mult1 = float(amp0) * 4.0 * np.pi / (k * k + (1.0 * float(shift0)) ** 2)
    mult2 = float(amp1) * 4.0 * np.pi / (k * k + (0.5 * float(shift1)) ** 2)
    w = np.full(K, 2.0)
    w[0] = 1.0
    scale = 1.0 / ((2.0 * np.pi * M / L) * (2.0 * np.pi))
    pref = scale * Cc * Cc / M
    g = np.stack([pref * w * deconv2 * mult1 * p * p,
                  pref * w * deconv2 * mult2 * p * p], axis=1)   # [K, 2]
    self2 = g.sum(axis=0)                                        # [2]

    gH = np.zeros((128, 4), np.float32)
    gH[0:K, 0:2] = g
    gH[K:128, 2:4] = g
    # a_blk = gH*csneg + HH2 with csneg = cs - 1024  =>  HH2 = 1024*gH + HH
    HH = np.zeros((128, 4), np.float32)
    HH[0, 0:2] = -self2
    HH[K, 2:4] = -self2
    HH = (1024.0 * gH.astype(np.float64) + HH).astype(np.float32)
    kst = np.zeros((6, 128), np.float32)
    kst[0:3, 0:K] = k[None, :]
    kst[3:6, K:128] = k[None, :]
    return gH, HH, kst.astype(np.float16)


def _split3(t):
    """t fp64 -> three fp16 arrays with t0+t1+t2 == t to ~2^-25."""
    t0 = t.astype(np.float16)
    r = t - t0.astype(np.float64)
    t1 = r.astype(np.float16)
    r = r - t1.astype(np.float64)
    t2 = r.astype(np.float16)
    return t0, t1, t2


def _build_program(debug=False):
    nc = bacc.Bacc(None, target_bir_lowering=False, debug=debug)
    # kt3 = [kst | t3] packed fp16; gHH = [gH | HH] packed f32
    kt3_in = nc.declare_dram_parameter("kt3", [6, 128 + N], F16, isOutput=False)
    gHH_in = nc.declare_dram_parameter("gHH", [128, 8], F32, isOutput=False)
    out_t = nc.declare_dram_parameter("out", [BPC, 2, N], F32, isOutput=True)

    with tile.TileContext(nc) as tc:
        import contextlib
        with contextlib.ExitStack() as ctx:
            pc = ctx.enter_context(tc.tile_pool(name="const", bufs=1))
            wp = ctx.enter_context(tc.tile_pool(name="work", bufs=1))
            ps_u = ctx.enter_context(tc.tile_pool(name="psu", bufs=1, space="PSUM"))
            ps_e = ctx.enter_context(tc.tile_pool(name="pse", bufs=1, space="PSUM"))

            kt3 = pc.tile([6, 128 + N], F16, tag="kt3")
            nc.sync.dma_start(kt3[:], kt3_in[:])
            gHH = pc.tile([128, 8], F32, tag="gHH")
            nc.scalar.dma_start(gHH[:], gHH_in[:])
            kst = kt3[:, 0:128]
            t3 = kt3[:, 128 : 128 + N]
            gH = gHH[:, 0:4]
            HH = gHH[:, 4:8]

            # phases u[j, n] = k_j * t_{b(j), n}  (j<64: batch0, j>=64: batch1)
            u = ps_u.tile([128, N], F32, tag="u")
            nc.tensor.matmul(u[:, 0:512], kst[:], t3[:, 0:512], start=True, stop=True)
            nc.tensor.matmul(u[:, 512:1024], kst[:], t3[:, 512:1024], start=True, stop=True)

            # negr = round(u) - u  (exact); sin(-2*pi*negr) = sin(2*pi*u)
            # v on the scalar engine (Copy applies in*scale + bias_imm)
            v = wp.tile([128, N], F32, tag="v")
            nc.scalar.activation(v[:], u[:], AF.Copy, bias=MAGIC)
            negr = wp.tile([128, N], F32, tag="negr")
            nc.vector.scalar_tensor_tensor(negr[:], v[:], MAGIC, u[:],
                                           ALU.subtract, ALU.subtract)

            h = wp.tile([128, N], F32, tag="h")
            nc.scalar.activation(h[:], negr[:], AF.Sin, scale=PI)
            ss = wp.tile([128, 1], F32, tag="ss")
            smat = wp.tile([128, N], F16, tag="smat")
            nc.scalar.activation(smat[:], negr[:], AF.Sin, scale=-2.0 * PI,
                                 accum_out=ss[:])
            # sin branch ready first: b_blk + its matmuls overlap the cos chain
            b_blk = wp.tile([128, 4], F16, tag="b_blk")
            nc.vector.tensor_scalar(b_blk[:], gH, ss[:], None, ALU.mult)

            # mneg = -2 sin^2(pi r) = cos(2 pi r) - 1;  csneg = sum(mneg) = cs - 1024
            mneg = wp.tile([128, N], F32, tag="mneg")
            csneg = wp.tile([128, 1], F32, tag="csneg")
            nc.vector.scalar_tensor_tensor(mneg[:], h[:], -2.0, h[:],
                                           ALU.mult, ALU.mult,
                                           accum_out=csneg[:])
            cmat = wp.tile([128, N], F16, tag="cmat")
            nc.vector.tensor_scalar(cmat[:], mneg[:], 1.0, None, ALU.add)

            # a_blk = gH*csneg + HH2 (self-energy + 1024*gH folded host-side)
            a_blk = wp.tile([128, 4], F16, tag="a_blk")
            nc.vector.scalar_tensor_tensor(a_blk[:], gH, csneg[:], HH,
                                           ALU.mult, ALU.add)

            # e[(b c), n] = a^T cos + b^T sin; sin first (ready earlier)
            e = ps_e.tile([4, N], F32, tag="e")
            nc.tensor.matmul(e[:, 0:512], b_blk[:], smat[:, 0:512], start=True, stop=False)
            nc.tensor.matmul(e[:, 512:1024], b_blk[:], smat[:, 512:1024], start=True, stop=False)
            nc.tensor.matmul(e[:, 0:512], a_blk[:], cmat[:, 0:512], start=False, stop=True)
            nc.tensor.matmul(e[:, 512:1024], a_blk[:], cmat[:, 512:1024], start=False, stop=True)

            es = wp.tile([4, N], F32, tag="es")
            nc.scalar.activation(es[:, 0:512], e[:, 0:512], AF.Copy)
            nc.vector.tensor_copy(es[:, 512:1024], e[:, 512:1024])
            # out rows (b0c0, b0c1, b1c0, b1c1) -> out[b, c, n] contiguous
            dst = out_t.rearrange("b c n -> (b c) n")
            nc.scalar.dma_start(dst[:, 0:512], es[:, 0:512])
            nc.gpsimd.dma_start(dst[:, 512:1024], es[:, 512:1024])
    return nc


def _make_in_maps(x, shift0, shift1, amp0, amp1):
    gH, HH, kst = _host_tables(shift0.reshape(-1)[0], shift1.reshape(-1)[0],
                               amp0.reshape(-1)[0], amp1.reshape(-1)[0])
    gHH = np.concatenate([gH, HH], axis=1)                      # [128, 8]
    t = np.asarray(x, np.float64) / (2.0 * np.pi)
    t0, t1, t2 = _split3(t)
    in_maps = []
    for c in range(NCORES):
        b0, b1 = BPC * c, BPC * c + 1
        t3 = np.stack([t0[b0], t1[b0], t2[b0], t0[b1], t1[b1], t2[b1]])
        kt3 = np.concatenate([kst, t3], axis=1)                 # [6, 1152]
        in_maps.append({"kt3": kt3, "gHH": gHH})
    return in_maps


def kernel(x, shift0, shift1, amp0, amp1):
    in_maps = _make_in_maps(x, shift0, shift1, amp0, amp1)
    nc = _build_program()
    nc.finalize()
    res = run_bass_kernel_spmd(nc, in_maps, list(range(NCORES)))
    # device emits [BPC, 2, N]; reference wants [B, N, 2]
    out = np.concatenate([res.results[c]["out"] for c in range(NCORES)], axis=0)
    return np.ascontiguousarray(out.transpose(0, 2, 1)).astype(np.float32)


# revision 21
# speedup vs baseline: 3.3746x; 1.0002x over previous
"""Trainium2 Bass kernel for nn_NUFFTLayerMultiChannelInitMixed.

Math: the reference's spread->FFT->filter->IFFT->energy pipeline is an exact
bilinear form in the analytic spectrum of the periodized Gaussians.  With the
M-aliased images dropped (their weight is exp(-tau*(k-M)^2) ~ 3e-5) the
energy reduces to a truncated cosine series in the K lowest modes:

  e_i[n] = sum_{k<K} g_ik * ( cs_k cos(k x_n) + ss_k sin(k x_n) ) - self_i
  cs_k   = sum_n cos(k x_n),   ss_k = sum_n sin(k x_n)
  g_ik   = pref * w_k * deconv^2_k * mult_ik * p_k^2   (host precomputed)

K=64 keeps rel err ~3e-4.  Each core packs its BPC=2 batches into the 128
partitions as [batch0: k=0..63 | batch1: k=0..63] with block-diagonal
stationaries, so the whole core workload is ONE [128, 1024] problem:

  phases  u = kst^T @ t3 on PE (fp16 3-term split of t keeps u exact to 2e-6)
  range reduction via the fp32 magic-rounding trick (2 DVE ops)
  sin/cos via two Sin activations (+ 1-2sin^2), accum_out gives cs/ss free
  e = a_blk^T @ cos + b_blk^T @ sin   (fp16, two 512-col PSUM banks)

self_i folds into the k=0 coefficient (cos(0)=1 row), so the PSUM result is
final and DMAs straight to DRAM.  No transposes, no collectives.
"""

import numpy as np

try:
    import concourse.bass as bass
except ImportError:
    import sys
    sys.path.insert(0, "/opt/trn_rl_repo")
    import concourse.bass as bass

import concourse.bacc as bacc
import concourse.mybir as mybir
from concourse import tile
from concourse.bass_utils import run_bass_kernel_spmd

F32 = mybir.dt.float32
F16 = mybir.dt.float16
AF = mybir.ActivationFunctionType
ALU = mybir.AluOpType

M = 2001
L = 2.0 * np.pi
TAU = 12.0 * (L / (2.0 * np.pi * M)) ** 2
K = 64                  # modes kept per batch
B_FULL, N = 16, 1024
NCORES = 8
BPC = B_FULL // NCORES  # 2 batches per core, packed into 2*K=128 partitions
MAGIC = 12582912.0      # 1.5 * 2^23: fl(u + MAGIC) - MAGIC = round-to-nearest(u)
PI = float(np.pi)


def _host_tables(shift0, shift1, amp0, amp1):
    """fp64 k-space tables: block-diag gain gH [128,4], self-energy HH [128,4],
    phase stationary kst [6,128] fp16."""
    k = np.arange(K, dtype=np.float64)
    p = np.exp(-TAU * k * k)
    Cc = (M / L) * np.sqrt(4.0 * np.pi * TAU)
    deconv2 = (np.pi / TAU) * np.exp(2.0 * TAU * k * k)
    mult1 = float(amp0) * 4.0 * np.pi / (k * k + (1.0 * float(shift0)) ** 2)
    mult2 = float(amp1) * 4.0 * np.pi / (k * k + (0.5 * float(shift1)) ** 2)
    w = np.full(K, 2.0)
    w[0] = 1.0
    scale = 1.0 / ((2.0 * np.pi * M / L) * (2.0 * np.pi))
    pref = scale * Cc * Cc / M
    g = np.stack([pref * w * deconv2 * mult1 * p * p,
                  pref * w * deconv2 * mult2 * p * p], axis=1)   # [K, 2]
    self2 = g.sum(axis=0)                                        # [2]

    gH = np.zeros((128, 4), np.float32)
    gH[0:K, 0:2] = g
    gH[K:128, 2:4] = g
    # a_blk = gH*csneg + HH2 with csneg = cs - 1024  =>  HH2 = 1024*gH + HH
    HH = np.zeros((128, 4), np.float32)
    HH[0, 0:2] = -self2
    HH[K, 2:4] = -self2
    HH = (1024.0 * gH.astype(np.float64) + HH).astype(np.float32)
    kst = np.zeros((6, 128), np.float32)
    kst[0:3, 0:K] = k[None, :]
    kst[3:6, K:128] = k[None, :]
    return gH, HH, kst.astype(np.float16)


def _split3(t):
    """t fp64 -> three fp16 arrays with t0+t1+t2 == t to ~2^-25."""
    t0 = t.astype(np.float16)
    r = t - t0.astype(np.float64)
    t1 = r.astype(np.float16)
    r = r - t1.astype(np.float64)
    t2 = r.astype(np.float16)
    return t0, t1, t2


def _build_program(debug=False):
    nc = bacc.Bacc(None, target_bir_lowering=False, debug=debug)
    # kt3 = [kst | t3] packed fp16; gHH = [gH | HH] packed f32
    kt3_in = nc.declare_dram_parameter("kt3", [6, 128 + N], F16, isOutput=False)
    gHH_in = nc.declare_dram_parameter("gHH", [128, 8], F32, isOutput=False)
    out_t = nc.declare_dram_parameter("out", [BPC, 2, N], F32, isOutput=True)

    with tile.TileContext(nc) as tc:
        import contextlib
        with contextlib.ExitStack() as ctx:
            pc = ctx.enter_context(tc.tile_pool(name="const", bufs=1))
            wp = ctx.enter_context(tc.tile_pool(name="work", bufs=1))
            ps_u = ctx.enter_context(tc.tile_pool(name="psu", bufs=1, space="PSUM"))
            ps_e = ctx.enter_context(tc.tile_pool(name="pse", bufs=1, space="PSUM"))

            kt3 = pc.tile([6, 128 + N], F16, tag="kt3")
            nc.sync.dma_start(kt3[:], kt3_in[:])
            gHH = pc.tile([128, 8], F32, tag="gHH")
            nc.scalar.dma_start(gHH[:], gHH_in[:])
            kst = kt3[:, 0:128]
            t3 = kt3[:, 128 : 128 + N]
            gH = gHH[:, 0:4]
            HH = gHH[:, 4:8]

            # phases u[j, n] = k_j * t_{b(j), n}  (j<64: batch0, j>=64: batch1)
            u = ps_u.tile([128, N], F32, tag="u")
            nc.tensor.matmul(u[:, 0:512], kst[:], t3[:, 0:512], start=True, stop=True)
            nc.tensor.matmul(u[:, 512:1024], kst[:], t3[:, 512:1024], start=True, stop=True)

            # negr = round(u) - u  (exact); sin(-2*pi*negr) = sin(2*pi*u)
            # all [128, N] stages split in 512-col halves for cross-engine overlap
            HALves = (slice(0, 512), slice(512, 1024))
            v = wp.tile([128, N], F32, tag="v")
            negr = wp.tile([128, N], F32, tag="negr")
            h = wp.tile([128, N], F32, tag="h")
            smat = wp.tile([128, N], F16, tag="smat")
            mneg = wp.tile([128, N], F32, tag="mneg")
            cmat = wp.tile([128, N], F16, tag="cmat")
            ss2 = wp.tile([128, 2], F32, tag="ss2")
            cn2 = wp.tile([128, 2], F32, tag="cn2")

            for i, sl in enumerate(HALves):
                nc.vector.tensor_scalar(v[:, sl], u[:, sl], MAGIC, None, ALU.add)
                nc.vector.scalar_tensor_tensor(negr[:, sl], v[:, sl], MAGIC,
                                               u[:, sl], ALU.subtract, ALU.subtract)
            for i, sl in enumerate(HALves):
                nc.scalar.activation(h[:, sl], negr[:, sl], AF.Sin, scale=PI)
            for i, sl in enumerate(HALves):
                nc.scalar.activation(smat[:, sl], negr[:, sl], AF.Sin,
                                     scale=-2.0 * PI, accum_out=ss2[:, i : i + 1])
            # mneg = -2 sin^2(pi r) = cos(2 pi r) - 1; csneg = sum(mneg) = cs - 1024
            for i, sl in enumerate(HALves):
                nc.vector.scalar_tensor_tensor(mneg[:, sl], h[:, sl], -2.0,
                                               h[:, sl], ALU.mult, ALU.mult,
                                               accum_out=cn2[:, i : i + 1])
            # sin branch ready first: b_blk + its matmuls overlap the cos chain
            ss = wp.tile([128, 1], F32, tag="ss")
            nc.vector.tensor_add(ss[:], ss2[:, 0:1], ss2[:, 1:2])
            b_blk = wp.tile([128, 4], F16, tag="b_blk")
            nc.vector.tensor_scalar(b_blk[:], gH, ss[:], None, ALU.mult)
            for i, sl in enumerate(HALves):
                nc.vector.tensor_scalar(cmat[:, sl], mneg[:, sl], 1.0, None, ALU.add)
            # a_blk = gH*csneg + HH2 (self-energy + 1024*gH folded host-side)
            csneg = wp.tile([128, 1], F32, tag="csneg")
            nc.vector.tensor_add(csneg[:], cn2[:, 0:1], cn2[:, 1:2])
            a_blk = wp.tile([128, 4], F16, tag="a_blk")
            nc.vector.scalar_tensor_tensor(a_blk[:], gH, csneg[:], HH,
                                           ALU.mult, ALU.add)

            # e[(b c), n] = a^T cos + b^T sin; sin first (ready earlier)
            e = ps_e.tile([4, N], F32, tag="e")
            nc.tensor.matmul(e[:, 0:512], b_blk[:], smat[:, 0:512], start=True, stop=False)
            nc.tensor.matmul(e[:, 512:1024], b_blk[:], smat[:, 512:1024], start=True, stop=False)
            nc.tensor.matmul(e[:, 0:512], a_blk[:], cmat[:, 0:512], start=False, stop=True)
            nc.tensor.matmul(e[:, 512:1024], a_blk[:], cmat[:, 512:1024], start=False, stop=True)

            es = wp.tile([4, N], F32, tag="es")
            nc.scalar.activation(es[:, 0:512], e[:, 0:512], AF.Copy)
            nc.vector.tensor_copy(es[:, 512:1024], e[:, 512:1024])
            # out rows (b0c0, b0c1, b1c0, b1c1) -> out[b, c, n] contiguous
            dst = out_t.rearrange("b c n -> (b c) n")
            nc.scalar.dma_start(dst[:, 0:512], es[:, 0:512])
            nc.sync.dma_start(dst[:, 512:1024], es[:, 512:1024])
    return nc


def _make_in_maps(x, shift0, shift1, amp0, amp1):
    gH, HH, kst = _host_tables(shift0.reshape(-1)[0], shift1.reshape(-1)[0],
                               amp0.reshape(-1)[0], amp1.reshape(-1)[0])
    gHH = np.concatenate([gH, HH], axis=1)                      # [128, 8]
    t = np.asarray(x, np.float64) / (2.0 * np.pi)
    t0, t1, t2 = _split3(t)
    in_maps = []
    for c in range(NCORES):
        b0, b1 = BPC * c, BPC * c + 1
        t3 = np.stack([t0[b0], t1[b0], t2[b0], t0[b1], t1[b1], t2[b1]])
        kt3 = np.concatenate([kst, t3], axis=1)                 # [6, 1152]
        in_maps.append({"kt3": kt3, "gHH": gHH})
    return in_maps


def kernel(x, shift0, shift1, amp0, amp1):
    in_maps = _make_in_maps(x, shift0, shift1, amp0, amp1)
    nc = _build_program()
    nc.finalize()
    res = run_bass_kernel_spmd(nc, in_maps, list(range(NCORES)))
    # device emits [BPC, 2, N]; reference wants [B, N, 2]
    out = np.concatenate([res.results[c]["out"] for c in range(NCORES)], axis=0)
    return np.ascontiguousarray(out.transpose(0, 2, 1)).astype(np.float32)


# revision 22
# speedup vs baseline: 3.4704x; 1.0284x over previous
"""Trainium2 Bass kernel for nn_NUFFTLayerMultiChannelInitMixed.

Math: the reference's spread->FFT->filter->IFFT->energy pipeline is an exact
bilinear form in the analytic spectrum of the periodized Gaussians.  With the
M-aliased images dropped (their weight is exp(-tau*(k-M)^2) ~ 3e-5) the
energy reduces to a truncated cosine series in the K lowest modes:

  e_i[n] = sum_{k<K} g_ik * ( cs_k cos(k x_n) + ss_k sin(k x_n) ) - self_i
  cs_k   = sum_n cos(k x_n),   ss_k = sum_n sin(k x_n)
  g_ik   = pref * w_k * deconv^2_k * mult_ik * p_k^2   (host precomputed)

K=64 keeps rel err ~3e-4.  Each core packs its BPC=2 batches into the 128
partitions as [batch0: k=0..63 | batch1: k=0..63] with block-diagonal
stationaries, so the whole core workload is ONE [128, 1024] problem:

  phases  u = kst^T @ t3 on PE (fp16 3-term split of t keeps u exact to 2e-6)
  range reduction via the fp32 magic-rounding trick (2 DVE ops)
  sin/cos via two Sin activations (+ 1-2sin^2), accum_out gives cs/ss free
  e = a_blk^T @ cos + b_blk^T @ sin   (fp16, two 512-col PSUM banks)

self_i folds into the k=0 coefficient (cos(0)=1 row), so the PSUM result is
final and DMAs straight to DRAM.  No transposes, no collectives.
"""

import numpy as np

try:
    import concourse.bass as bass
except ImportError:
    import sys
    sys.path.insert(0, "/opt/trn_rl_repo")
    import concourse.bass as bass

import concourse.bacc as bacc
import concourse.mybir as mybir
from concourse import tile
from concourse.bass_utils import run_bass_kernel_spmd

F32 = mybir.dt.float32
F16 = mybir.dt.float16
AF = mybir.ActivationFunctionType
ALU = mybir.AluOpType

M = 2001
L = 2.0 * np.pi
TAU = 12.0 * (L / (2.0 * np.pi * M)) ** 2
K = 64                  # modes kept per batch
B_FULL, N = 16, 1024
NCORES = 8
BPC = B_FULL // NCORES  # 2 batches per core, packed into 2*K=128 partitions
MAGIC = 12582912.0      # 1.5 * 2^23: fl(u + MAGIC) - MAGIC = round-to-nearest(u)
PI = float(np.pi)


def _host_tables(shift0, shift1, amp0, amp1):
    """fp64 k-space tables: block-diag gain gH [128,4], self-energy HH [128,4],
    phase stationary kst [6,128] fp16."""
    k = np.arange(K, dtype=np.float64)
    p = np.exp(-TAU * k * k)
    Cc = (M / L) * np.sqrt(4.0 * np.pi * TAU)
    deconv2 = (np.pi / TAU) * np.exp(2.0 * TAU * k * k)
    mult1 = float(amp0) * 4.0 * np.pi / (k * k + (1.0 * float(shift0)) ** 2)
    mult2 = float(amp1) * 4.0 * np.pi / (k * k + (0.5 * float(shift1)) ** 2)
    w = np.full(K, 2.0)
    w[0] = 1.0
    scale = 1.0 / ((2.0 * np.pi * M / L) * (2.0 * np.pi))
    pref = scale * Cc * Cc / M
    g = np.stack([pref * w * deconv2 * mult1 * p * p,
                  pref * w * deconv2 * mult2 * p * p], axis=1)   # [K, 2]
    self2 = g.sum(axis=0)                                        # [2]

    gH = np.zeros((128, 4), np.float32)
    gH[0:K, 0:2] = g
    gH[K:128, 2:4] = g
    # a_blk = gH*csneg + HH2 with csneg = cs - 1024  =>  HH2 = 1024*gH + HH
    HH = np.zeros((128, 4), np.float32)
    HH[0, 0:2] = -self2
    HH[K, 2:4] = -self2
    HH = (1024.0 * gH.astype(np.float64) + HH).astype(np.float32)
    kst = np.zeros((6, 128), np.float32)
    kst[0:3, 0:K] = k[None, :]
    kst[3:6, K:128] = k[None, :]
    return gH, HH, kst.astype(np.float16)


def _split3(t):
    """t fp64 -> three fp16 arrays with t0+t1+t2 == t to ~2^-25."""
    t0 = t.astype(np.float16)
    r = t - t0.astype(np.float64)
    t1 = r.astype(np.float16)
    r = r - t1.astype(np.float64)
    t2 = r.astype(np.float16)
    return t0, t1, t2


def _build_program(debug=False):
    nc = bacc.Bacc(None, target_bir_lowering=False, debug=debug)
    # kt3 = [kst | t3] packed fp16; gHH = [gH | HH] packed f32
    kt3_in = nc.declare_dram_parameter("kt3", [6, 128 + N], F16, isOutput=False)
    gHH_in = nc.declare_dram_parameter("gHH", [128, 8], F32, isOutput=False)
    out_t = nc.declare_dram_parameter("out", [BPC, 2, N], F32, isOutput=True)

    with tile.TileContext(nc) as tc:
        import contextlib
        with contextlib.ExitStack() as ctx:
            pc = ctx.enter_context(tc.tile_pool(name="const", bufs=1))
            wp = ctx.enter_context(tc.tile_pool(name="work", bufs=1))
            ps_u = ctx.enter_context(tc.tile_pool(name="psu", bufs=1, space="PSUM"))
            ps_e = ctx.enter_context(tc.tile_pool(name="pse", bufs=1, space="PSUM"))

            kt3 = pc.tile([6, 128 + N], F16, tag="kt3")
            nc.sync.dma_start(kt3[:], kt3_in[:])
            gHH = pc.tile([128, 8], F32, tag="gHH")
            nc.scalar.dma_start(gHH[:], gHH_in[:])
            kst = kt3[:, 0:128]
            t3 = kt3[:, 128 : 128 + N]
            gH = gHH[:, 0:4]
            HH = gHH[:, 4:8]

            # phases u[j, n] = k_j * t_{b(j), n}  (j<64: batch0, j>=64: batch1)
            u = ps_u.tile([128, N], F32, tag="u")
            nc.tensor.matmul(u[:, 0:512], kst[:], t3[:, 0:512], start=True, stop=True)
            nc.tensor.matmul(u[:, 512:1024], kst[:], t3[:, 512:1024], start=True, stop=True)

            # negr = round(u) - u  (exact); sin(-2*pi*negr) = sin(2*pi*u)
            # all [128, N] stages split in 512-col halves for cross-engine overlap
            HALves = (slice(0, 512), slice(512, 1024))
            v = wp.tile([128, N], F32, tag="v")
            negr = wp.tile([128, N], F32, tag="negr")
            h = wp.tile([128, N], F16, tag="h")
            smat = wp.tile([128, N], F16, tag="smat")
            mneg = wp.tile([128, N], F16, tag="mneg")
            cmat = wp.tile([128, N], F16, tag="cmat")
            ss2 = wp.tile([128, 2], F32, tag="ss2")
            cn2 = wp.tile([128, 2], F32, tag="cn2")

            for i, sl in enumerate(HALves):
                nc.vector.tensor_scalar(v[:, sl], u[:, sl], MAGIC, None, ALU.add)
                nc.vector.scalar_tensor_tensor(negr[:, sl], v[:, sl], MAGIC,
                                               u[:, sl], ALU.subtract, ALU.subtract)
            for i, sl in enumerate(HALves):
                nc.scalar.activation(h[:, sl], negr[:, sl], AF.Sin, scale=PI)
            for i, sl in enumerate(HALves):
                nc.scalar.activation(smat[:, sl], negr[:, sl], AF.Sin,
                                     scale=-2.0 * PI, accum_out=ss2[:, i : i + 1])
            # mneg = -2 sin^2(pi r) = cos(2 pi r) - 1; csneg = sum(mneg) = cs - 1024
            for i, sl in enumerate(HALves):
                nc.vector.scalar_tensor_tensor(mneg[:, sl], h[:, sl], -2.0,
                                               h[:, sl], ALU.mult, ALU.mult,
                                               accum_out=cn2[:, i : i + 1])
            # cos-side coefficients first: a-matmuls can start while the sin
            # side (ss accumulators) is still draining on the scalar engine
            csneg = wp.tile([128, 1], F32, tag="csneg")
            nc.vector.tensor_add(csneg[:], cn2[:, 0:1], cn2[:, 1:2])
            # a_blk = gH*csneg + HH2 (self-energy + 1024*gH folded host-side)
            a_blk = wp.tile([128, 4], F16, tag="a_blk")
            nc.vector.scalar_tensor_tensor(a_blk[:], gH, csneg[:], HH,
                                           ALU.mult, ALU.add)
            for i, sl in enumerate(HALves):
                nc.vector.tensor_scalar(cmat[:, sl], mneg[:, sl], 1.0, None, ALU.add)
            ss = wp.tile([128, 1], F32, tag="ss")
            nc.vector.tensor_add(ss[:], ss2[:, 0:1], ss2[:, 1:2])
            b_blk = wp.tile([128, 4], F16, tag="b_blk")
            nc.vector.tensor_scalar(b_blk[:], gH, ss[:], None, ALU.mult)

            # e[(b c), n] = a^T cos + b^T sin; cos first (ready earlier)
            e = ps_e.tile([4, N], F32, tag="e")
            nc.tensor.matmul(e[:, 0:512], a_blk[:], cmat[:, 0:512], start=True, stop=False)
            nc.tensor.matmul(e[:, 512:1024], a_blk[:], cmat[:, 512:1024], start=True, stop=False)
            nc.tensor.matmul(e[:, 0:512], b_blk[:], smat[:, 0:512], start=False, stop=True)
            nc.tensor.matmul(e[:, 512:1024], b_blk[:], smat[:, 512:1024], start=False, stop=True)

            es = wp.tile([4, N], F32, tag="es")
            nc.scalar.activation(es[:, 0:512], e[:, 0:512], AF.Copy)
            nc.vector.tensor_copy(es[:, 512:1024], e[:, 512:1024])
            # out rows (b0c0, b0c1, b1c0, b1c1) -> out[b, c, n] contiguous
            dst = out_t.rearrange("b c n -> (b c) n")
            nc.scalar.dma_start(dst[:, 0:512], es[:, 0:512])
            nc.sync.dma_start(dst[:, 512:1024], es[:, 512:1024])
    return nc


def _make_in_maps(x, shift0, shift1, amp0, amp1):
    gH, HH, kst = _host_tables(shift0.reshape(-1)[0], shift1.reshape(-1)[0],
                               amp0.reshape(-1)[0], amp1.reshape(-1)[0])
    gHH = np.concatenate([gH, HH], axis=1)                      # [128, 8]
    t = np.asarray(x, np.float64) / (2.0 * np.pi)
    t0, t1, t2 = _split3(t)
    in_maps = []
    for c in range(NCORES):
        b0, b1 = BPC * c, BPC * c + 1
        t3 = np.stack([t0[b0], t1[b0], t2[b0], t0[b1], t1[b1], t2[b1]])
        kt3 = np.concatenate([kst, t3], axis=1)                 # [6, 1152]
        in_maps.append({"kt3": kt3, "gHH": gHH})
    return in_maps


def kernel(x, shift0, shift1, amp0, amp1):
    in_maps = _make_in_maps(x, shift0, shift1, amp0, amp1)
    nc = _build_program()
    nc.finalize()
    res = run_bass_kernel_spmd(nc, in_maps, list(range(NCORES)))
    # device emits [BPC, 2, N]; reference wants [B, N, 2]
    out = np.concatenate([res.results[c]["out"] for c in range(NCORES)], axis=0)
    return np.ascontiguousarray(out.transpose(0, 2, 1)).astype(np.float32)


# revision 29
# speedup vs baseline: 3.6412x; 1.0492x over previous
"""Trainium2 Bass kernel for nn_NUFFTLayerMultiChannelInitMixed.

Math: the reference's spread->FFT->filter->IFFT->energy pipeline is an exact
bilinear form in the analytic spectrum of the periodized Gaussians.  With the
M-aliased images dropped (their weight is exp(-tau*(k-M)^2) ~ 3e-5) the
energy reduces to a truncated cosine series in the K lowest modes:

  e_i[n] = sum_{k<K} g_ik * ( cs_k cos(k x_n) + ss_k sin(k x_n) ) - self_i
  cs_k   = sum_n cos(k x_n),   ss_k = sum_n sin(k x_n)
  g_ik   = pref * w_k * deconv^2_k * mult_ik * p_k^2   (host precomputed)

K=32 keeps rel err ~4e-4 (gate is 2e-2).  Each core handles BPC=2 batches;
partition p = 64*nhalf + 32*batch + k packs BOTH 512-point halves of BOTH
batches into the 128 partitions, so every elementwise stage is a single
[128, 512] instruction:

  phases   u = kst12^T @ t3p on PE (fp16 3-term split of t keeps u exact)
  range    v = u + MAGIC; negr = (v - MAGIC) - u   (exact fp32 rounding trick)
  trig     h = sin(pi*negr); smat = sin(2*pi*u) via Sin ACT (accum -> ss)
           mneg = -2h^2 = cos - 1 (accum -> cs-1024); cmat = mneg + 1
  energy   e = a^T cos + b^T sin, four block-masked fp16 stationaries
           (per n-half), self-energy and the +1024 fold baked host-side

No transposes, no collectives; output written [b, c, n] and transposed on
the host.
"""

import numpy as np

try:
    import concourse.bass as bass
except ImportError:
    import sys
    sys.path.insert(0, "/opt/trn_rl_repo")
    import concourse.bass as bass

import concourse.bacc as bacc
import concourse.mybir as mybir
from concourse import tile
from concourse.bass_utils import run_bass_kernel_spmd

F32 = mybir.dt.float32
F16 = mybir.dt.float16
AF = mybir.ActivationFunctionType
ALU = mybir.AluOpType

M = 2001
L = 2.0 * np.pi
TAU = 12.0 * (L / (2.0 * np.pi * M)) ** 2
K = 32                  # modes kept per batch
B_FULL, N = 16, 1024
NH = 512                # points per n-half
NCORES = 8
BPC = B_FULL // NCORES
MAGIC = 12582912.0      # 1.5 * 2^23: fl(u + MAGIC) - MAGIC = round-to-nearest(u)
PI = float(np.pi)


def _host_tables(shift0, shift1, amp0, amp1):
    """fp64 k-space tables for the packed layout p = 64*nh + 32*b + k.

    Returns kst12 [12,128] fp16 (phase stationary, rows r = 6*nh + 3*b + i)
    and gHHa [128,16] f32 = [gM_h0 | gM_h1 | HHa_h0 | HHa_h1]."""
    k = np.arange(K, dtype=np.float64)
    p = np.exp(-TAU * k * k)
    Cc = (M / L) * np.sqrt(4.0 * np.pi * TAU)
    deconv2 = (np.pi / TAU) * np.exp(2.0 * TAU * k * k)
    mult1 = float(amp0) * 4.0 * np.pi / (k * k + (1.0 * float(shift0)) ** 2)
    mult2 = float(amp1) * 4.0 * np.pi / (k * k + (0.5 * float(shift1)) ** 2)
    w = np.full(K, 2.0)
    w[0] = 1.0
    scale = 1.0 / ((2.0 * np.pi * M / L) * (2.0 * np.pi))
    pref = scale * Cc * Cc / M
    g = np.stack([pref * w * deconv2 * mult1 * p * p,
                  pref * w * deconv2 * mult2 * p * p], axis=1)   # [K, 2]
    self2 = g.sum(axis=0)

    pp = np.arange(128)
    nh_p, b_p, k_p = pp // 64, (pp // 32) % 2, pp % 32
    gM = np.zeros((2, 128, 4))
    HHa = np.zeros((2, 128, 4))
    for nh in range(2):
        rows = np.nonzero(nh_p == nh)[0]
        for r in rows:
            for c in range(2):
                col = 2 * b_p[r] + c
                gM[nh, r, col] = g[k_p[r], c]
                HHa[nh, r, col] = -self2[c] if k_p[r] == 0 else 0.0
    gHHa = np.concatenate([gM[0], gM[1], HHa[0], HHa[1]],
                          axis=1).astype(np.float32)             # [128, 16]

    kst12 = np.zeros((12, 128), np.float32)
    for r in range(12):
        r_nh, r_b = r // 6, (r // 3) % 2
        sel = (nh_p == r_nh) & (b_p == r_b)
        kst12[r, sel] = k_p[sel]

    # fold matrix: cs2[p'] = sum_p Fold[p, p'] csh[p] adds the two n-halves
    fold = (pp[:, None] % 64 == pp[None, :] % 64).astype(np.float16)
    return kst12.astype(np.float16), gHHa, fold


def _split3(t):
    """t fp64 -> three fp16 arrays with t0+t1+t2 == t to ~2^-25."""
    t0 = t.astype(np.float16)
    r = t - t0.astype(np.float64)
    t1 = r.astype(np.float16)
    r = r - t1.astype(np.float64)
    t2 = r.astype(np.float16)
    return t0, t1, t2


def _build_program(debug=False):
    nc = bacc.Bacc(None, target_bir_lowering=False, debug=debug)
    # kt3 = [kst12 | t3p] packed fp16; gHHa = [gM_h0|gM_h1|HHa_h0|HHa_h1] f32
    kt3_in = nc.declare_dram_parameter("kt3", [12, 128 + NH], F16, isOutput=False)
    gHH_in = nc.declare_dram_parameter("gHH", [128, 16], F32, isOutput=False)
    fold_in = nc.declare_dram_parameter("fold", [128, 128], F16, isOutput=False)
    out_t = nc.declare_dram_parameter("out", [BPC, 2, N], F32, isOutput=True)

    with tile.TileContext(nc) as tc:
        import contextlib
        with contextlib.ExitStack() as ctx:
            pc = ctx.enter_context(tc.tile_pool(name="const", bufs=1))
            wp = ctx.enter_context(tc.tile_pool(name="work", bufs=1))
            ps_u = ctx.enter_context(tc.tile_pool(name="psu", bufs=1, space="PSUM"))
            ps_e = ctx.enter_context(tc.tile_pool(name="pse", bufs=1, space="PSUM"))

            kt3 = pc.tile([12, 128 + NH], F16, tag="kt3")
            nc.sync.dma_start(kt3[:], kt3_in[:])
            gHH = pc.tile([128, 16], F32, tag="gHH")
            nc.scalar.dma_start(gHH[:], gHH_in[:])
            fold = pc.tile([128, 128], F16, tag="fold")
            nc.sync.dma_start(fold[:], fold_in[:])
            kst = kt3[:, 0:128]
            t3p = kt3[:, 128 : 128 + NH]

            # phases u[p, n'] = k(p) * t_{b(p)}[512*nh(p) + n']
            u = ps_u.tile([128, NH], F32, tag="u")
            nc.tensor.matmul(u[:], kst, t3p, start=True, stop=True)

            # negr = round(u) - u (exact); sin(-2*pi*negr) = sin(2*pi*u)
            v = wp.tile([128, NH], F32, tag="v")
            nc.vector.tensor_scalar(v[:], u[:], MAGIC, None, ALU.add)
            negr = wp.tile([128, NH], F32, tag="negr")
            nc.vector.scalar_tensor_tensor(negr[:], v[:], MAGIC, u[:],
                                           ALU.subtract, ALU.subtract)

            h = wp.tile([128, NH], F16, tag="h")
            nc.scalar.activation(h[:], negr[:], AF.Sin, scale=PI)
            sscn = wp.tile([128, 1], F32, tag="sscn")
            smat = wp.tile([128, NH], F16, tag="smat")
            nc.scalar.activation(smat[:], negr[:], AF.Sin, scale=-2.0 * PI,
                                 accum_out=sscn[:])

            # mneg = -2 sin^2(pi r) = cos - 1
            mneg = wp.tile([128, NH], F16, tag="mneg")
            nc.vector.scalar_tensor_tensor(mneg[:], h[:], -2.0, h[:],
                                           ALU.mult, ALU.mult)
            # cmat = mneg + 1 = cos;  accum csh = sum over this n-half
            csh = wp.tile([128, 1], F32, tag="csh")
            cmat = wp.tile([128, NH], F16, tag="cmat")
            nc.vector.tensor_scalar(cmat[:], mneg[:], 1.0, 0.0, ALU.add,
                                    ALU.add, accum_out=csh[:])

            # fold the two n-halves on the PE: cs2[p'] = csh[p'&63] + csh[64+(p'&63)]
            cns16 = wp.tile([128, 2], F16, tag="cns16")
            nc.vector.tensor_copy(cns16[:, 0:1], csh[:])
            nc.vector.tensor_copy(cns16[:, 1:2], sscn[:])
            cs2 = ps_u.tile([128, 2], F32, tag="cs2")
            nc.tensor.matmul(cs2[:], fold[:], cns16[:], start=True, stop=True)
            cssb = wp.tile([128, 2], F32, tag="cssb")
            nc.vector.tensor_copy(cssb[:], cs2[:])

            # cos-side stationaries first; their matmuls overlap the sin side
            a_h0 = wp.tile([128, 4], F16, tag="a_h0")
            nc.vector.scalar_tensor_tensor(a_h0[:], gHH[:, 0:4], cssb[:, 0:1],
                                           gHH[:, 8:12], ALU.mult, ALU.add)
            a_h1 = wp.tile([128, 4], F16, tag="a_h1")
            nc.vector.scalar_tensor_tensor(a_h1[:], gHH[:, 4:8], cssb[:, 0:1],
                                           gHH[:, 12:16], ALU.mult, ALU.add)
            b_h0 = wp.tile([128, 4], F16, tag="b_h0")
            nc.vector.tensor_scalar(b_h0[:], gHH[:, 0:4], cssb[:, 1:2], None, ALU.mult)
            b_h1 = wp.tile([128, 4], F16, tag="b_h1")
            nc.vector.tensor_scalar(b_h1[:], gHH[:, 4:8], cssb[:, 1:2], None, ALU.mult)

            # e[(b c), n] per n-half; bank nh = a_hnh^T cmat + b_hnh^T smat
            e = ps_e.tile([4, N], F32, tag="e")
            nc.tensor.matmul(e[:, 0:512], a_h0[:], cmat[:], start=True, stop=False)
            nc.tensor.matmul(e[:, 512:1024], a_h1[:], cmat[:], start=True, stop=False)
            nc.tensor.matmul(e[:, 0:512], b_h0[:], smat[:], start=False, stop=True)
            nc.tensor.matmul(e[:, 512:1024], b_h1[:], smat[:], start=False, stop=True)

            es = wp.tile([4, N], F32, tag="es")
            nc.scalar.activation(es[:, 0:512], e[:, 0:512], AF.Copy)
            nc.vector.tensor_copy(es[:, 512:1024], e[:, 512:1024])
            # out rows (b0c0, b0c1, b1c0, b1c1) -> out[b, c, n] contiguous
            dst = out_t.rearrange("b c n -> (b c) n")
            nc.scalar.dma_start(dst[:, 0:512], es[:, 0:512])
            nc.sync.dma_start(dst[:, 512:1024], es[:, 512:1024])
    return nc


def _make_in_maps(x, shift0, shift1, amp0, amp1):
    kst12, gHHa, fold = _host_tables(shift0.reshape(-1)[0], shift1.reshape(-1)[0],
                                     amp0.reshape(-1)[0], amp1.reshape(-1)[0])
    t = np.asarray(x, np.float64) / (2.0 * np.pi)
    t0, t1, t2 = _split3(t)
    in_maps = []
    for c in range(NCORES):
        b0, b1 = BPC * c, BPC * c + 1
        t3p = np.zeros((12, NH), np.float16)
        for r_nh in range(2):
            for r_b, bb in ((0, b0), (1, b1)):
                for i, tt in enumerate((t0, t1, t2)):
                    t3p[6 * r_nh + 3 * r_b + i] = tt[bb, NH * r_nh : NH * (r_nh + 1)]
        kt3 = np.concatenate([kst12, t3p], axis=1)               # [12, 640]
        in_maps.append({"kt3": kt3, "gHH": gHHa, "fold": fold})
    return in_maps


def kernel(x, shift0, shift1, amp0, amp1):
    in_maps = _make_in_maps(x, shift0, shift1, amp0, amp1)
    nc = _build_program()
    nc.finalize()
    res = run_bass_kernel_spmd(nc, in_maps, list(range(NCORES)))
    # device emits [BPC, 2, N]; reference wants [B, N, 2]
    out = np.concatenate([res.results[c]["out"] for c in range(NCORES)], axis=0)
    return np.ascontiguousarray(out.transpose(0, 2, 1)).astype(np.float32)


# revision 32
# speedup vs baseline: 3.6926x; 1.0141x over previous
"""Trainium2 Bass kernel for nn_NUFFTLayerMultiChannelInitMixed.

Math: the reference's spread->FFT->filter->IFFT->energy pipeline is an exact
bilinear form in the analytic spectrum of the periodized Gaussians.  With the
M-aliased images dropped (their weight is exp(-tau*(k-M)^2) ~ 3e-5) the
energy reduces to a truncated cosine series in the K lowest modes:

  e_i[n] = sum_{k<K} g_ik * ( cs_k cos(k x_n) + ss_k sin(k x_n) ) - self_i
  cs_k   = sum_n cos(k x_n),   ss_k = sum_n sin(k x_n)
  g_ik   = pref * w_k * deconv^2_k * mult_ik * p_k^2   (host precomputed)

K=32 keeps rel err ~4e-4 (gate is 2e-2).  Each core handles BPC=2 batches;
partition p = 64*nhalf + 32*batch + k packs BOTH 512-point halves of BOTH
batches into the 128 partitions, so every elementwise stage is a single
[128, 512] instruction:

  phases   u = kst12^T @ t3p on PE (fp16 3-term split of t keeps u exact)
  range    v = u + MAGIC; negr = (v - MAGIC) - u   (exact fp32 rounding trick)
  trig     h = sin(pi*negr); smat = sin(2*pi*u) via Sin ACT (accum -> ss)
           mneg = -2h^2 = cos - 1 (accum -> cs-1024); cmat = mneg + 1
  energy   e = a^T cos + b^T sin, four block-masked fp16 stationaries
           (per n-half), self-energy and the +1024 fold baked host-side

No transposes, no collectives; output written [b, c, n] and transposed on
the host.
"""

import numpy as np

try:
    import concourse.bass as bass
except ImportError:
    import sys
    sys.path.insert(0, "/opt/trn_rl_repo")
    import concourse.bass as bass

import concourse.bacc as bacc
import concourse.mybir as mybir
from concourse import tile
from concourse.bass_utils import run_bass_kernel_spmd

F32 = mybir.dt.float32
F16 = mybir.dt.float16
AF = mybir.ActivationFunctionType
ALU = mybir.AluOpType

M = 2001
L = 2.0 * np.pi
TAU = 12.0 * (L / (2.0 * np.pi * M)) ** 2
K = 32                  # modes kept per batch
B_FULL, N = 16, 1024
NH = 512                # points per n-half
NCORES = 8
BPC = B_FULL // NCORES
MAGIC = 12582912.0      # 1.5 * 2^23: fl(u + MAGIC) - MAGIC = round-to-nearest(u)
PI = float(np.pi)


def _host_tables(shift0, shift1, amp0, amp1):
    """fp64 k-space tables for the packed layout p = 64*nh + 32*b + k.

    Returns kst12 [12,128] fp16 (phase stationary, rows r = 6*nh + 3*b + i)
    and gHHa [128,16] f32 = [gM_h0 | gM_h1 | HHa_h0 | HHa_h1]."""
    k = np.arange(K, dtype=np.float64)
    p = np.exp(-TAU * k * k)
    Cc = (M / L) * np.sqrt(4.0 * np.pi * TAU)
    deconv2 = (np.pi / TAU) * np.exp(2.0 * TAU * k * k)
    mult1 = float(amp0) * 4.0 * np.pi / (k * k + (1.0 * float(shift0)) ** 2)
    mult2 = float(amp1) * 4.0 * np.pi / (k * k + (0.5 * float(shift1)) ** 2)
    w = np.full(K, 2.0)
    w[0] = 1.0
    scale = 1.0 / ((2.0 * np.pi * M / L) * (2.0 * np.pi))
    pref = scale * Cc * Cc / M
    g = np.stack([pref * w * deconv2 * mult1 * p * p,
                  pref * w * deconv2 * mult2 * p * p], axis=1)   # [K, 2]
    self2 = g.sum(axis=0)

    pp = np.arange(128)
    nh_p, b_p, k_p = pp // 64, (pp // 32) % 2, pp % 32
    gM = np.zeros((2, 128, 4))
    HHa = np.zeros((2, 128, 4))
    for nh in range(2):
        rows = np.nonzero(nh_p == nh)[0]
        for r in rows:
            for c in range(2):
                col = 2 * b_p[r] + c
                gM[nh, r, col] = g[k_p[r], c]
                # cs arrives as csneg = cs - 1024: fold 1024*g into the bias
                HHa[nh, r, col] = 1024.0 * g[k_p[r], c] - (
                    self2[c] if k_p[r] == 0 else 0.0)
    gHHa = np.concatenate([gM[0], gM[1], HHa[0], HHa[1]],
                          axis=1).astype(np.float32)             # [128, 16]

    kst12 = np.zeros((12, 128), np.float32)
    for r in range(12):
        r_nh, r_b = r // 6, (r // 3) % 2
        sel = (nh_p == r_nh) & (b_p == r_b)
        kst12[r, sel] = k_p[sel]

    # fold matrix: cs2[p'] = sum_p Fold[p, p'] csh[p] adds the two n-halves
    fold = (pp[:, None] % 64 == pp[None, :] % 64).astype(np.float16)
    return kst12.astype(np.float16), gHHa, fold


def _split3(t):
    """t fp64 -> three fp16 arrays with t0+t1+t2 == t to ~2^-25."""
    t0 = t.astype(np.float16)
    r = t - t0.astype(np.float64)
    t1 = r.astype(np.float16)
    r = r - t1.astype(np.float64)
    t2 = r.astype(np.float16)
    return t0, t1, t2


def _build_program(debug=False):
    nc = bacc.Bacc(None, target_bir_lowering=False, debug=debug)
    # kt3 = [kst12 | t3p] packed fp16; gHHa = [gM_h0|gM_h1|HHa_h0|HHa_h1] f32
    kt3_in = nc.declare_dram_parameter("kt3", [12, 128 + NH], F16, isOutput=False)
    gHH_in = nc.declare_dram_parameter("gHH", [128, 16], F32, isOutput=False)
    fold_in = nc.declare_dram_parameter("fold", [128, 128], F16, isOutput=False)
    out_t = nc.declare_dram_parameter("out", [BPC, 2, N], F32, isOutput=True)

    with tile.TileContext(nc) as tc:
        import contextlib
        with contextlib.ExitStack() as ctx:
            pc = ctx.enter_context(tc.tile_pool(name="const", bufs=1))
            wp = ctx.enter_context(tc.tile_pool(name="work", bufs=1))
            ps_u = ctx.enter_context(tc.tile_pool(name="psu", bufs=1, space="PSUM"))
            ps_e = ctx.enter_context(tc.tile_pool(name="pse", bufs=1, space="PSUM"))

            kt3 = pc.tile([12, 128 + NH], F16, tag="kt3")
            nc.sync.dma_start(kt3[:], kt3_in[:])
            gHH = pc.tile([128, 16], F32, tag="gHH")
            nc.scalar.dma_start(gHH[:], gHH_in[:])
            fold = pc.tile([128, 128], F16, tag="fold")
            nc.sync.dma_start(fold[:], fold_in[:])
            kst = kt3[:, 0:128]
            t3p = kt3[:, 128 : 128 + NH]

            # phases u[p, n'] = k(p) * t_{b(p)}[512*nh(p) + n']
            u = ps_u.tile([128, NH], F32, tag="u")
            nc.tensor.matmul(u[:], kst, t3p, start=True, stop=True)

            # negr = round(u) - u (exact); sin(-2*pi*negr) = sin(2*pi*u)
            v = wp.tile([128, NH], F32, tag="v")
            nc.vector.tensor_scalar(v[:], u[:], MAGIC, None, ALU.add)
            negr = wp.tile([128, NH], F32, tag="negr")
            nc.vector.scalar_tensor_tensor(negr[:], v[:], MAGIC, u[:],
                                           ALU.subtract, ALU.subtract)

            h = wp.tile([128, NH], F16, tag="h")
            nc.scalar.activation(h[:], negr[:], AF.Sin, scale=PI)
            sscn = wp.tile([128, 1], F32, tag="sscn")
            smat = wp.tile([128, NH], F16, tag="smat")
            nc.scalar.activation(smat[:], negr[:], AF.Sin, scale=-2.0 * PI,
                                 accum_out=sscn[:])

            # mneg = -2 sin^2(pi r) = cos - 1;  accum csn = sum(mneg) = cs_half - 512
            mneg = wp.tile([128, NH], F16, tag="mneg")
            csn = wp.tile([128, 1], F32, tag="csn")
            nc.vector.scalar_tensor_tensor(mneg[:], h[:], -2.0, h[:],
                                           ALU.mult, ALU.mult, accum_out=csn[:])
            # fold the two n-halves on the PE: cs2[p'] = csn[p'&63] + csn[64+(p'&63)]
            cns16 = wp.tile([128, 2], F16, tag="cns16")
            nc.vector.tensor_copy(cns16[:, 0:1], csn[:])
            nc.vector.tensor_copy(cns16[:, 1:2], sscn[:])
            cmat = wp.tile([128, NH], F16, tag="cmat")
            nc.vector.tensor_scalar(cmat[:], mneg[:], 1.0, None, ALU.add)
            cs2 = ps_u.tile([128, 2], F32, tag="cs2")
            nc.tensor.matmul(cs2[:], fold[:], cns16[:], start=True, stop=True)

            # cos-side stationaries first; their matmuls overlap the sin side
            a_h0 = wp.tile([128, 4], F16, tag="a_h0")
            nc.vector.scalar_tensor_tensor(a_h0[:], gHH[:, 0:4], cs2[:, 0:1],
                                           gHH[:, 8:12], ALU.mult, ALU.add)
            a_h1 = wp.tile([128, 4], F16, tag="a_h1")
            nc.vector.scalar_tensor_tensor(a_h1[:], gHH[:, 4:8], cs2[:, 0:1],
                                           gHH[:, 12:16], ALU.mult, ALU.add)
            b_h0 = wp.tile([128, 4], F16, tag="b_h0")
            nc.vector.tensor_scalar(b_h0[:], gHH[:, 0:4], cs2[:, 1:2], None, ALU.mult)
            b_h1 = wp.tile([128, 4], F16, tag="b_h1")
            nc.vector.tensor_scalar(b_h1[:], gHH[:, 4:8], cs2[:, 1:2], None, ALU.mult)

            # e[(b c), n] per n-half; bank nh = a_hnh^T cmat + b_hnh^T smat
            e = ps_e.tile([4, N], F32, tag="e")
            nc.tensor.matmul(e[:, 0:512], a_h0[:], cmat[:], start=True, stop=False)
            nc.tensor.matmul(e[:, 512:1024], a_h1[:], cmat[:], start=True, stop=False)
            nc.tensor.matmul(e[:, 0:512], b_h0[:], smat[:], start=False, stop=True)
            nc.tensor.matmul(e[:, 512:1024], b_h1[:], smat[:], start=False, stop=True)

            es = wp.tile([4, N], F32, tag="es")
            nc.scalar.activation(es[:, 0:512], e[:, 0:512], AF.Copy)
            nc.vector.tensor_copy(es[:, 512:1024], e[:, 512:1024])
            # out rows (b0c0, b0c1, b1c0, b1c1) -> out[b, c, n] contiguous
            dst = out_t.rearrange("b c n -> (b c) n")
            nc.gpsimd.dma_start(dst[:, 0:512], es[:, 0:512])
            nc.sync.dma_start(dst[:, 512:1024], es[:, 512:1024])
    return nc


def _make_in_maps(x, shift0, shift1, amp0, amp1):
    kst12, gHHa, fold = _host_tables(shift0.reshape(-1)[0], shift1.reshape(-1)[0],
                                     amp0.reshape(-1)[0], amp1.reshape(-1)[0])
    t = np.asarray(x, np.float64) / (2.0 * np.pi)
    t0, t1, t2 = _split3(t)
    in_maps = []
    for c in range(NCORES):
        b0, b1 = BPC * c, BPC * c + 1
        t3p = np.zeros((12, NH), np.float16)
        for r_nh in range(2):
            for r_b, bb in ((0, b0), (1, b1)):
                for i, tt in enumerate((t0, t1, t2)):
                    t3p[6 * r_nh + 3 * r_b + i] = tt[bb, NH * r_nh : NH * (r_nh + 1)]
        kt3 = np.concatenate([kst12, t3p], axis=1)               # [12, 640]
        in_maps.append({"kt3": kt3, "gHH": gHHa, "fold": fold})
    return in_maps


def kernel(x, shift0, shift1, amp0, amp1):
    in_maps = _make_in_maps(x, shift0, shift1, amp0, amp1)
    nc = _build_program()
    nc.finalize()
    res = run_bass_kernel_spmd(nc, in_maps, list(range(NCORES)))
    # device emits [BPC, 2, N]; reference wants [B, N, 2]
    out = np.concatenate([res.results[c]["out"] for c in range(NCORES)], axis=0)
    return np.ascontiguousarray(out.transpose(0, 2, 1)).astype(np.float32)


# revision 34
# speedup vs baseline: 3.7572x; 1.0175x over previous
"""Trainium2 Bass kernel for nn_NUFFTLayerMultiChannelInitMixed.

Math: the reference's spread->FFT->filter->IFFT->energy pipeline is an exact
bilinear form in the analytic spectrum of the periodized Gaussians.  With the
M-aliased images dropped (their weight is exp(-tau*(k-M)^2) ~ 3e-5) the
energy reduces to a truncated cosine series in the K lowest modes:

  e_i[n] = sum_{k<K} g_ik * ( cs_k cos(k x_n) + ss_k sin(k x_n) ) - self_i
  cs_k   = sum_n cos(k x_n),   ss_k = sum_n sin(k x_n)
  g_ik   = pref * w_k * deconv^2_k * mult_ik * p_k^2   (host precomputed)

K=32 keeps rel err ~4e-4 (gate is 2e-2).  Each core handles BPC=2 batches;
partition p = 64*nhalf + 32*batch + k packs BOTH 512-point halves of BOTH
batches into the 128 partitions, so every elementwise stage is a single
[128, 512] instruction:

  phases   u = kst12^T @ t3p on PE (fp16 3-term split of t keeps u exact)
  range    v = u + MAGIC; negr = (v - MAGIC) - u   (exact fp32 rounding trick)
  trig     h = sin(pi*negr); smat = sin(2*pi*u) via Sin ACT (accum -> ss)
           mneg = -2h^2 = cos - 1 (accum -> cs-1024); cmat = mneg + 1
  energy   e = a^T cos + b^T sin, four block-masked fp16 stationaries
           (per n-half), self-energy and the +1024 fold baked host-side

No transposes, no collectives; output written [b, c, n] and transposed on
the host.
"""

import numpy as np

try:
    import concourse.bass as bass
except ImportError:
    import sys
    sys.path.insert(0, "/opt/trn_rl_repo")
    import concourse.bass as bass

import concourse.bacc as bacc
import concourse.mybir as mybir
from concourse import tile
from concourse.bass_utils import run_bass_kernel_spmd

F32 = mybir.dt.float32
F16 = mybir.dt.float16
AF = mybir.ActivationFunctionType
ALU = mybir.AluOpType

M = 2001
L = 2.0 * np.pi
TAU = 12.0 * (L / (2.0 * np.pi * M)) ** 2
K = 32                  # modes kept per batch
B_FULL, N = 16, 1024
NH = 512                # points per n-half
NCORES = 8
BPC = B_FULL // NCORES
MAGIC = 12582912.0      # 1.5 * 2^23: fl(u + MAGIC) - MAGIC = round-to-nearest(u)
PI = float(np.pi)


def _host_tables(shift0, shift1, amp0, amp1):
    """fp64 k-space tables for the packed layout p = 64*nh + 32*b + k.

    Returns kst12 [12,128] fp16 (phase stationary, rows r = 6*nh + 3*b + i)
    and gHHa [128,16] f32 = [gM_h0 | gM_h1 | HHa_h0 | HHa_h1]."""
    k = np.arange(K, dtype=np.float64)
    p = np.exp(-TAU * k * k)
    Cc = (M / L) * np.sqrt(4.0 * np.pi * TAU)
    deconv2 = (np.pi / TAU) * np.exp(2.0 * TAU * k * k)
    mult1 = float(amp0) * 4.0 * np.pi / (k * k + (1.0 * float(shift0)) ** 2)
    mult2 = float(amp1) * 4.0 * np.pi / (k * k + (0.5 * float(shift1)) ** 2)
    w = np.full(K, 2.0)
    w[0] = 1.0
    scale = 1.0 / ((2.0 * np.pi * M / L) * (2.0 * np.pi))
    pref = scale * Cc * Cc / M
    g = np.stack([pref * w * deconv2 * mult1 * p * p,
                  pref * w * deconv2 * mult2 * p * p], axis=1)   # [K, 2]
    self2 = g.sum(axis=0)

    pp = np.arange(128)
    nh_p, b_p, k_p = pp // 64, (pp // 32) % 2, pp % 32
    gM = np.zeros((2, 128, 4))
    HHa = np.zeros((2, 128, 4))
    for nh in range(2):
        rows = np.nonzero(nh_p == nh)[0]
        for r in rows:
            for c in range(2):
                col = 2 * b_p[r] + c
                gM[nh, r, col] = g[k_p[r], c]
                # cs arrives as csneg = cs - 1024: fold 1024*g into the bias
                HHa[nh, r, col] = 1024.0 * g[k_p[r], c] - (
                    self2[c] if k_p[r] == 0 else 0.0)
    gHHa = np.concatenate([gM[0], gM[1], HHa[0], HHa[1]],
                          axis=1).astype(np.float32)             # [128, 16]

    kst12 = np.zeros((12, 128), np.float32)
    for r in range(12):
        r_nh, r_b = r // 6, (r // 3) % 2
        sel = (nh_p == r_nh) & (b_p == r_b)
        kst12[r, sel] = k_p[sel]

    # fold matrix: cs2[p'] = sum_p Fold[p, p'] csh[p] adds the two n-halves
    fold = (pp[:, None] % 64 == pp[None, :] % 64).astype(np.float16)
    return kst12.astype(np.float16), gHHa, fold


def _split3(t):
    """t fp64 -> three fp16 arrays with t0+t1+t2 == t to ~2^-25."""
    t0 = t.astype(np.float16)
    r = t - t0.astype(np.float64)
    t1 = r.astype(np.float16)
    r = r - t1.astype(np.float64)
    t2 = r.astype(np.float16)
    return t0, t1, t2


def _build_program(debug=False):
    nc = bacc.Bacc(None, target_bir_lowering=False, debug=debug)
    # kt3 = [kst12 | t3p] packed fp16; gHHa = [gM_h0|gM_h1|HHa_h0|HHa_h1] f32
    kt3_in = nc.declare_dram_parameter("kt3", [12, 128 + NH], F16, isOutput=False)
    gHH_in = nc.declare_dram_parameter("gHH", [128, 16], F32, isOutput=False)
    fold_in = nc.declare_dram_parameter("fold", [128, 128], F16, isOutput=False)
    out_t = nc.declare_dram_parameter("out", [BPC, 2, N], F32, isOutput=True)

    with tile.TileContext(nc) as tc:
        import contextlib
        with contextlib.ExitStack() as ctx:
            pc = ctx.enter_context(tc.tile_pool(name="const", bufs=1))
            wp = ctx.enter_context(tc.tile_pool(name="work", bufs=1))
            ps_u = ctx.enter_context(tc.tile_pool(name="psu", bufs=1, space="PSUM"))
            ps_e = ctx.enter_context(tc.tile_pool(name="pse", bufs=1, space="PSUM"))

            kt3 = pc.tile([12, 128 + NH], F16, tag="kt3")
            nc.sync.dma_start(kt3[:], kt3_in[:])
            gHH = pc.tile([128, 16], F32, tag="gHH")
            nc.scalar.dma_start(gHH[:], gHH_in[:])
            fold = pc.tile([128, 128], F16, tag="fold")
            nc.sync.dma_start(fold[:], fold_in[:])
            kst = kt3[:, 0:128]
            t3p = kt3[:, 128 : 128 + NH]

            # phases u[p, n'] = k(p) * t_{b(p)}[512*nh(p) + n']
            u = ps_u.tile([128, NH], F32, tag="u")
            nc.tensor.matmul(u[:], kst, t3p, start=True, stop=True)

            # negr = round(u) - u (exact); sin(-2*pi*negr) = sin(2*pi*u)
            v = wp.tile([128, NH], F32, tag="v")
            nc.vector.tensor_scalar(v[:], u[:], MAGIC, None, ALU.add)
            negr = wp.tile([128, NH], F32, tag="negr")
            nc.vector.scalar_tensor_tensor(negr[:], v[:], MAGIC, u[:],
                                           ALU.subtract, ALU.subtract)

            h = wp.tile([128, NH], F16, tag="h")
            nc.scalar.activation(h[:], negr[:], AF.Sin, scale=PI)
            sscn = wp.tile([128, 1], F32, tag="sscn")
            smat = wp.tile([128, NH], F16, tag="smat")
            nc.scalar.activation(smat[:], negr[:], AF.Sin, scale=-2.0 * PI,
                                 accum_out=sscn[:])

            # mneg = -2 sin^2(pi r) = cos - 1;  accum csn = sum(mneg) = cs_half - 512
            mneg = wp.tile([128, NH], F16, tag="mneg")
            csn = wp.tile([128, 1], F32, tag="csn")
            nc.vector.scalar_tensor_tensor(mneg[:], h[:], -2.0, h[:],
                                           ALU.mult, ALU.mult, accum_out=csn[:])
            # fold the two n-halves on the PE: cs2[p'] = csn[p'&63] + csn[64+(p'&63)]
            cns16 = wp.tile([128, 2], F16, tag="cns16")
            nc.vector.tensor_copy(cns16[:, 0:1], csn[:])
            nc.vector.tensor_copy(cns16[:, 1:2], sscn[:])
            cmat = wp.tile([128, NH], F16, tag="cmat")
            nc.vector.tensor_scalar(cmat[:], mneg[:], 1.0, None, ALU.add)
            cs2 = ps_u.tile([128, 2], F32, tag="cs2")
            nc.tensor.matmul(cs2[:], fold[:], cns16[:], start=True, stop=True)

            # cos-side stationaries first; their matmuls overlap the sin side
            a_h0 = wp.tile([128, 4], F16, tag="a_h0")
            nc.vector.scalar_tensor_tensor(a_h0[:], gHH[:, 0:4], cs2[:, 0:1],
                                           gHH[:, 8:12], ALU.mult, ALU.add)
            a_h1 = wp.tile([128, 4], F16, tag="a_h1")
            nc.vector.scalar_tensor_tensor(a_h1[:], gHH[:, 4:8], cs2[:, 0:1],
                                           gHH[:, 12:16], ALU.mult, ALU.add)
            b_h0 = wp.tile([128, 4], F16, tag="b_h0")
            nc.vector.tensor_scalar(b_h0[:], gHH[:, 0:4], cs2[:, 1:2], None, ALU.mult)
            b_h1 = wp.tile([128, 4], F16, tag="b_h1")
            nc.vector.tensor_scalar(b_h1[:], gHH[:, 4:8], cs2[:, 1:2], None, ALU.mult)

            # e[(b c), n] per n-half; bank nh = a_hnh^T cmat + b_hnh^T smat
            e = ps_e.tile([4, N], F32, tag="e")
            nc.tensor.matmul(e[:, 0:512], a_h0[:], cmat[:], start=True, stop=False)
            nc.tensor.matmul(e[:, 512:1024], a_h1[:], cmat[:], start=True, stop=False)
            nc.tensor.matmul(e[:, 0:512], b_h0[:], smat[:], start=False, stop=True)
            nc.tensor.matmul(e[:, 512:1024], b_h1[:], smat[:], start=False, stop=True)

            es = wp.tile([4, N], F32, tag="es")
            nc.scalar.activation(es[:, 0:512], e[:, 0:512], AF.Copy)
            nc.vector.tensor_copy(es[:, 512:1024], e[:, 512:1024])
            # out rows (b0c0, b0c1, b1c0, b1c1) -> out[b, c, n] contiguous
            dst = out_t.rearrange("b c n -> (b c) n")
            nc.gpsimd.dma_start(dst[:, 0:512], es[:, 0:512])
            nc.sync.dma_start(dst[:, 512:1024], es[:, 512:1024])
    return nc


def _make_in_maps(x, shift0, shift1, amp0, amp1):
    kst12, gHHa, fold = _host_tables(shift0.reshape(-1)[0], shift1.reshape(-1)[0],
                                     amp0.reshape(-1)[0], amp1.reshape(-1)[0])
    t = np.asarray(x, np.float64) / (2.0 * np.pi)
    t0, t1, t2 = _split3(t)
    in_maps = []
    for c in range(NCORES):
        b0, b1 = BPC * c, BPC * c + 1
        t3p = np.zeros((12, NH), np.float16)
        for r_nh in range(2):
            for r_b, bb in ((0, b0), (1, b1)):
                for i, tt in enumerate((t0, t1, t2)):
                    t3p[6 * r_nh + 3 * r_b + i] = tt[bb, NH * r_nh : NH * (r_nh + 1)]
        kt3 = np.concatenate([kst12, t3p], axis=1)               # [12, 640]
        in_maps.append({"kt3": kt3, "gHH": gHHa, "fold": fold})
    return in_maps


def kernel(x, shift0, shift1, amp0, amp1):
    in_maps = _make_in_maps(x, shift0, shift1, amp0, amp1)
    nc = _build_program()
    nc.finalize()
    res = run_bass_kernel_spmd(nc, in_maps, list(range(NCORES)))
    # device emits [BPC, 2, N]; reference wants [B, N, 2]
    out = np.concatenate([res.results[c]["out"] for c in range(NCORES)], axis=0)
    return np.ascontiguousarray(out.transpose(0, 2, 1)).astype(np.float32)


# revision 36
# speedup vs baseline: 3.7901x; 1.0088x over previous
"""Trainium2 Bass kernel for nn_NUFFTLayerMultiChannelInitMixed.

Math: the reference's spread->FFT->filter->IFFT->energy pipeline is an exact
bilinear form in the analytic spectrum of the periodized Gaussians.  With the
M-aliased images dropped (their weight is exp(-tau*(k-M)^2) ~ 3e-5) the
energy reduces to a truncated cosine series in the K lowest modes:

  e_i[n] = sum_{k<K} g_ik * ( cs_k cos(k x_n) + ss_k sin(k x_n) ) - self_i
  cs_k   = sum_n cos(k x_n),   ss_k = sum_n sin(k x_n)
  g_ik   = pref * w_k * deconv^2_k * mult_ik * p_k^2   (host precomputed)

K=32 keeps rel err ~4e-4 (gate is 2e-2).  Each core handles BPC=2 batches;
partition p = 64*nhalf + 32*batch + k packs BOTH 512-point halves of BOTH
batches into the 128 partitions, so every elementwise stage is a single
[128, 512] instruction:

  phases   u = kst12^T @ t3p on PE (fp16 3-term split of t keeps u exact)
  range    v = u + MAGIC; negr = (v - MAGIC) - u   (exact fp32 rounding trick)
  trig     h = sin(pi*negr); smat = sin(2*pi*u) via Sin ACT (accum -> ss)
           mneg = -2h^2 = cos - 1 (accum -> cs-1024); cmat = mneg + 1
  energy   e = a^T cos + b^T sin, four block-masked fp16 stationaries
           (per n-half), self-energy and the +1024 fold baked host-side

No transposes, no collectives; output written [b, c, n] and transposed on
the host.
"""

import numpy as np

try:
    import concourse.bass as bass
except ImportError:
    import sys
    sys.path.insert(0, "/opt/trn_rl_repo")
    import concourse.bass as bass

import concourse.bacc as bacc
import concourse.mybir as mybir
from concourse import tile
from concourse.bass_utils import run_bass_kernel_spmd

F32 = mybir.dt.float32
F16 = mybir.dt.float16
AF = mybir.ActivationFunctionType
ALU = mybir.AluOpType

M = 2001
L = 2.0 * np.pi
TAU = 12.0 * (L / (2.0 * np.pi * M)) ** 2
K = 32                  # modes kept per batch
B_FULL, N = 16, 1024
NH = 512                # points per n-half
NCORES = 8
BPC = B_FULL // NCORES
MAGIC = 12582912.0      # 1.5 * 2^23: fl(u + MAGIC) - MAGIC = round-to-nearest(u)
PI = float(np.pi)


def _host_tables(shift0, shift1, amp0, amp1):
    """fp64 k-space tables for the packed layout p = 64*nh + 32*b + k.

    Returns kst12 [12,128] fp16 (phase stationary, rows r = 6*nh + 3*b + i)
    and gHHa [128,16] f32 = [gM_h0 | gM_h1 | HHa_h0 | HHa_h1]."""
    k = np.arange(K, dtype=np.float64)
    p = np.exp(-TAU * k * k)
    Cc = (M / L) * np.sqrt(4.0 * np.pi * TAU)
    deconv2 = (np.pi / TAU) * np.exp(2.0 * TAU * k * k)
    mult1 = float(amp0) * 4.0 * np.pi / (k * k + (1.0 * float(shift0)) ** 2)
    mult2 = float(amp1) * 4.0 * np.pi / (k * k + (0.5 * float(shift1)) ** 2)
    w = np.full(K, 2.0)
    w[0] = 1.0
    scale = 1.0 / ((2.0 * np.pi * M / L) * (2.0 * np.pi))
    pref = scale * Cc * Cc / M
    g = np.stack([pref * w * deconv2 * mult1 * p * p,
                  pref * w * deconv2 * mult2 * p * p], axis=1)   # [K, 2]
    self2 = g.sum(axis=0)

    pp = np.arange(128)
    nh_p, b_p, k_p = pp // 64, (pp // 32) % 2, pp % 32
    gM = np.zeros((2, 128, 4))
    HHa = np.zeros((2, 128, 4))
    for nh in range(2):
        rows = np.nonzero(nh_p == nh)[0]
        for r in rows:
            for c in range(2):
                col = 2 * b_p[r] + c
                gM[nh, r, col] = g[k_p[r], c]
                # cs arrives as csneg = cs - 1024: fold 1024*g into the bias
                HHa[nh, r, col] = 1024.0 * g[k_p[r], c] - (
                    self2[c] if k_p[r] == 0 else 0.0)
    gHHa = np.concatenate([gM[0], gM[1], HHa[0], HHa[1]],
                          axis=1).astype(np.float32)             # [128, 16]

    kst12 = np.zeros((8, 128), np.float32)
    for r in range(8):
        r_nh, r_b = r // 4, (r // 2) % 2
        sel = (nh_p == r_nh) & (b_p == r_b)
        kst12[r, sel] = k_p[sel]

    # fold matrix: cs2[p'] = sum_p Fold[p, p'] csh[p] adds the two n-halves
    fold = (pp[:, None] % 64 == pp[None, :] % 64).astype(np.float16)
    return kst12.astype(np.float16), gHHa, fold


def _split3(t):
    """t fp64 -> three fp16 arrays with t0+t1+t2 == t to ~2^-25."""
    t0 = t.astype(np.float16)
    r = t - t0.astype(np.float64)
    t1 = r.astype(np.float16)
    r = r - t1.astype(np.float64)
    t2 = r.astype(np.float16)
    return t0, t1, t2


def _build_program(debug=False):
    nc = bacc.Bacc(None, target_bir_lowering=False, debug=debug)
    # kt3 = [kst12 | t3p] packed fp16; gHHa = [gM_h0|gM_h1|HHa_h0|HHa_h1] f32
    kt3_in = nc.declare_dram_parameter("kt3", [8, 128 + NH], F16, isOutput=False)
    gHH_in = nc.declare_dram_parameter("gHH", [128, 16], F32, isOutput=False)
    fold_in = nc.declare_dram_parameter("fold", [128, 128], F16, isOutput=False)
    out_t = nc.declare_dram_parameter("out", [BPC, 2, N], F32, isOutput=True)

    with tile.TileContext(nc) as tc:
        import contextlib
        with contextlib.ExitStack() as ctx:
            pc = ctx.enter_context(tc.tile_pool(name="const", bufs=1))
            wp = ctx.enter_context(tc.tile_pool(name="work", bufs=1))
            ps_u = ctx.enter_context(tc.tile_pool(name="psu", bufs=1, space="PSUM"))
            ps_e = ctx.enter_context(tc.tile_pool(name="pse", bufs=1, space="PSUM"))

            kt3 = pc.tile([8, 128 + NH], F16, tag="kt3")
            nc.sync.dma_start(kt3[:], kt3_in[:])
            gHH = pc.tile([128, 16], F32, tag="gHH")
            nc.scalar.dma_start(gHH[:], gHH_in[:])
            fold = pc.tile([128, 128], F16, tag="fold")
            nc.sync.dma_start(fold[:], fold_in[:])
            kst = kt3[:, 0:128]
            t3p = kt3[:, 128 : 128 + NH]

            # phases u[p, n'] = k(p) * t_{b(p)}[512*nh(p) + n']
            u = ps_u.tile([128, NH], F32, tag="u")
            nc.tensor.matmul(u[:], kst, t3p, start=True, stop=True)

            # negr = round(u) - u (exact); sin(-2*pi*negr) = sin(2*pi*u)
            v = wp.tile([128, NH], F32, tag="v")
            nc.vector.tensor_scalar(v[:], u[:], MAGIC, None, ALU.add)
            negr = wp.tile([128, NH], F32, tag="negr")
            nc.vector.scalar_tensor_tensor(negr[:], v[:], MAGIC, u[:],
                                           ALU.subtract, ALU.subtract)

            h = wp.tile([128, NH], F16, tag="h")
            nc.scalar.activation(h[:], negr[:], AF.Sin, scale=PI)
            sscn = wp.tile([128, 1], F32, tag="sscn")
            smat = wp.tile([128, NH], F16, tag="smat")
            nc.scalar.activation(smat[:], negr[:], AF.Sin, scale=-2.0 * PI,
                                 accum_out=sscn[:])

            # mneg = -2 sin^2(pi r) = cos - 1;  accum csn = sum(mneg) = cs_half - 512
            mneg = wp.tile([128, NH], F16, tag="mneg")
            csn = wp.tile([128, 1], F32, tag="csn")
            nc.vector.scalar_tensor_tensor(mneg[:], h[:], -2.0, h[:],
                                           ALU.mult, ALU.mult, accum_out=csn[:])
            # fold the two n-halves on the PE: cs2[p'] = csn[p'&63] + csn[64+(p'&63)]
            cns16 = wp.tile([128, 2], F16, tag="cns16")
            nc.vector.tensor_copy(cns16[:, 0:1], csn[:])
            nc.vector.tensor_copy(cns16[:, 1:2], sscn[:])
            cs2 = ps_u.tile([128, 2], F32, tag="cs2")
            nc.tensor.matmul(cs2[:], fold[:], cns16[:], start=True, stop=True)
            cmat = wp.tile([128, NH], F16, tag="cmat")
            nc.vector.tensor_scalar(cmat[:], mneg[:], 1.0, None, ALU.add)

            # cos-side stationaries first; their matmuls overlap the sin side
            a_h0 = wp.tile([128, 4], F16, tag="a_h0")
            nc.vector.scalar_tensor_tensor(a_h0[:], gHH[:, 0:4], cs2[:, 0:1],
                                           gHH[:, 8:12], ALU.mult, ALU.add)
            a_h1 = wp.tile([128, 4], F16, tag="a_h1")
            nc.vector.scalar_tensor_tensor(a_h1[:], gHH[:, 4:8], cs2[:, 0:1],
                                           gHH[:, 12:16], ALU.mult, ALU.add)
            b_h0 = wp.tile([128, 4], F16, tag="b_h0")
            nc.vector.tensor_scalar(b_h0[:], gHH[:, 0:4], cs2[:, 1:2], None, ALU.mult)
            b_h1 = wp.tile([128, 4], F16, tag="b_h1")
            nc.vector.tensor_scalar(b_h1[:], gHH[:, 4:8], cs2[:, 1:2], None, ALU.mult)

            # e[(b c), n] per n-half; bank nh = a_hnh^T cmat + b_hnh^T smat
            e = ps_e.tile([4, N], F32, tag="e")
            nc.tensor.matmul(e[:, 0:512], a_h0[:], cmat[:], start=True, stop=False)
            nc.tensor.matmul(e[:, 512:1024], a_h1[:], cmat[:], start=True, stop=False)
            nc.tensor.matmul(e[:, 0:512], b_h0[:], smat[:], start=False, stop=True)
            nc.tensor.matmul(e[:, 512:1024], b_h1[:], smat[:], start=False, stop=True)

            es = wp.tile([4, N], F32, tag="es")
            nc.scalar.activation(es[:, 0:512], e[:, 0:512], AF.Copy)
            nc.vector.tensor_copy(es[:, 512:1024], e[:, 512:1024])
            # out rows (b0c0, b0c1, b1c0, b1c1) -> out[b, c, n] contiguous
            dst = out_t.rearrange("b c n -> (b c) n")
            nc.gpsimd.dma_start(dst[:, 0:512], es[:, 0:512])
            nc.sync.dma_start(dst[:, 512:1024], es[:, 512:1024])
    return nc


def _make_in_maps(x, shift0, shift1, amp0, amp1):
    kst12, gHHa, fold = _host_tables(shift0.reshape(-1)[0], shift1.reshape(-1)[0],
                                     amp0.reshape(-1)[0], amp1.reshape(-1)[0])
    t = np.asarray(x, np.float64) / (2.0 * np.pi)
    t0, t1, t2 = _split3(t)
    in_maps = []
    for c in range(NCORES):
        b0, b1 = BPC * c, BPC * c + 1
        t3p = np.zeros((8, NH), np.float16)
        for r_nh in range(2):
            for r_b, bb in ((0, b0), (1, b1)):
                for i, tt in enumerate((t0, t1)):
                    t3p[4 * r_nh + 2 * r_b + i] = tt[bb, NH * r_nh : NH * (r_nh + 1)]
        kt3 = np.concatenate([kst12, t3p], axis=1)               # [12, 640]
        in_maps.append({"kt3": kt3, "gHH": gHHa, "fold": fold})
    return in_maps


def kernel(x, shift0, shift1, amp0, amp1):
    in_maps = _make_in_maps(x, shift0, shift1, amp0, amp1)
    nc = _build_program()
    nc.finalize()
    res = run_bass_kernel_spmd(nc, in_maps, list(range(NCORES)))
    # device emits [BPC, 2, N]; reference wants [B, N, 2]
    out = np.concatenate([res.results[c]["out"] for c in range(NCORES)], axis=0)
    return np.ascontiguousarray(out.transpose(0, 2, 1)).astype(np.float32)
